# revision 1
# baseline (speedup 1.0000x reference)
"""GAT classifier on 8 trn2 NeuronCores (Bass/Tile).

Sharding: 1D node partition (6250 nodes/core); edges assigned to the core
owning their dst node, sorted by dst into 128-node chunks. Per chunk the
segmented softmax + weighted aggregation is done with PSUM-accumulated
"selection matmuls": S_x[e, j] = (slot_e == j) * x_e built by a dual-op
tensor_scalar against an iota tile, then U[j, :] += S_x^T @ [h_rows | 1].
exp() moves to the dense phase via the factorization
  exp(leaky_relu(as + ad)) = max(u*v, u'*v'),  u = e^as, u' = e^{0.2 as},
                                               v = e^ad, v' = e^{0.2 ad}.
Layer tables (node rows packed [h0|1|h1|1|u,u']) are AllGathered across
cores between layers; pooling partial sums are AllReduced.
"""
import math
import os
import sys
from contextlib import ExitStack
from dataclasses import dataclass

import numpy as np

for _p in ("/opt/trn_rl_repo", "/root/.axon_site/_ro/trn_rl_repo"):
    if os.path.isdir(_p) and _p not in sys.path:
        sys.path.insert(0, _p)

import concourse.bacc as bacc
import concourse.bass as bass
import concourse.mybir as mybir
import concourse.tile as tile
from concourse.tile import add_dep_helper
from concourse.bass_utils import run_bass_kernel_spmd
from concourse.masks import make_identity

P = 128
AF = mybir.ActivationFunctionType
ALU = mybir.AluOpType
F32 = mybir.dt.float32
I16 = mybir.dt.int16


@dataclass
class Cfg:
    N: int = 50000
    E0: int = 800000
    IN: int = 128
    HID: int = 64
    G: int = 512
    CORES: int = 8
    # filled by plan()
    NPC: int = 0
    CH: int = 0
    HALF: int = 0
    SEC_LO: int = 0
    SEC_HI: int = 0
    EC: int = 0
    T: int = 0
    T_LO: int = 0
    G_CH: int = 2

    @property
    def NCH(self):  # padded per-core node count
        return self.CH * P


def plan_cfg(N, E0, G, CORES=8):
    c = Cfg(N=N, E0=E0, G=G, CORES=CORES)
    assert N % CORES == 0
    c.NPC = N // CORES
    c.CH = math.ceil(c.NPC / P)
    c.HALF = ((N // 2) + 127) & ~127  # half-split point for int16 indices
    assert c.HALF < 32768 * 2 and (N - c.HALF) <= 32767 and c.HALF <= 32767
    return c


# ----------------------------------------------------------------- host prep

def prep_edges(cfg, src, dst):
    """Per-core edge arrays. Returns list of dicts + fills cfg.SEC_*/EC/T."""
    owner = dst // cfg.NPC
    per_core = []
    maxlo = maxhi = 0
    for c in range(cfg.CORES):
        m = owner == c
        s = src[m]
        dl = dst[m] - c * cfg.NPC
        chunk = dl >> 7
        half = (s >= cfg.HALF).astype(np.int64)
        order = np.lexsort((s, half, chunk))
        s, dl, chunk, half = s[order], dl[order], chunk[order], half[order]
        key = chunk * 2 + half
        cnt = np.bincount(key, minlength=cfg.CH * 2).reshape(cfg.CH, 2)
        maxlo = max(maxlo, int(cnt[:, 0].max()))
        maxhi = max(maxhi, int(cnt[:, 1].max()))
        per_core.append((s, dl, chunk, half, cnt))
    cfg.SEC_LO = ((maxlo + 127) & ~127) or P
    cfg.SEC_HI = ((maxhi + 127) & ~127) or P
    cfg.EC = cfg.SEC_LO + cfg.SEC_HI
    cfg.T = cfg.EC // P
    cfg.T_LO = cfg.SEC_LO // P

    out = []
    for c in range(cfg.CORES):
        s, dl, chunk, half, cnt = per_core[c]
        gl = np.zeros((cfg.CH, cfg.SEC_LO), np.int16)
        gh = np.zeros((cfg.CH, cfg.SEC_HI), np.int16)
        vi = np.zeros((cfg.CH, cfg.EC), np.int16)
        sl = np.full((cfg.CH, cfg.EC), 300.0, np.float32)
        ofs = np.zeros(cfg.CH * 2 + 1, np.int64)
        np.cumsum(cnt.reshape(-1), out=ofs[1:])
        for k in range(cfg.CH):
            nlo, nhi = int(cnt[k, 0]), int(cnt[k, 1])
            a = ofs[2 * k]
            gl[k, :nlo] = s[a:a + nlo]
            vi[k, :nlo] = dl[a:a + nlo]
            sl[k, :nlo] = (dl[a:a + nlo] & 127).astype(np.float32)
            b = ofs[2 * k + 1]
            gh[k, :nhi] = s[b:b + nhi] - cfg.HALF
            vi[k, cfg.SEC_LO:cfg.SEC_LO + nhi] = dl[b:b + nhi]
            sl[k, cfg.SEC_LO:cfg.SEC_LO + nhi] = (dl[b:b + nhi] & 127).astype(np.float32)

        def wrap16(a):  # idx i -> [i % 16, i // 16], replicated over 8 groups
            w = a.reshape(-1, 16).T.copy()
            return np.tile(w, (8, 1)).astype(np.int16)

        out.append(dict(
            gl=wrap16(gl), gh=wrap16(gh), vi=wrap16(vi),
            slot=sl.reshape(cfg.CH * cfg.T, P).T.copy(),
        ))
    return out


def prep_inputs(cfg, x, edge_index, batch, W1, a_src1, a_dst1, W2, a_src2, a_dst2, fcW):
    N, CORES, NPC, CH = cfg.N, cfg.CORES, cfg.NPC, cfg.CH
    src = np.concatenate([edge_index[0], np.arange(N)]).astype(np.int64)
    dst = np.concatenate([edge_index[1], np.arange(N)]).astype(np.int64)
    edges = prep_edges(cfg, src, dst)

    H = 2
    HID = cfg.HID
    rhs1 = np.zeros((cfg.IN, H * HID + 4), np.float32)
    rhs1[:, :H * HID] = W1
    for h in range(H):
        rhs1[:, H * HID + h] = W1[:, h * HID:(h + 1) * HID] @ a_src1[h]
        rhs1[:, H * HID + 2 + h] = W1[:, h * HID:(h + 1) * HID] @ a_dst1[h]
    rhs2 = np.zeros((H * HID, HID + 2), np.float32)
    rhs2[:, :HID] = W2
    rhs2[:, HID] = W2 @ a_src2[0]
    rhs2[:, HID + 1] = W2 @ a_dst2[0]

    iota128 = np.tile(np.arange(P, dtype=np.float32), (P, 1))
    iota512 = np.tile(np.arange(cfg.G, dtype=np.float32), (P, 1))
    cnt = np.bincount(batch, minlength=cfg.G).astype(np.float32)
    invc = 1.0 / np.maximum(cnt, 1.0)
    invc_b = np.tile(invc, (HID, 1)).astype(np.float32)

    xT = np.zeros((cfg.IN, CORES * cfg.NCH), np.float32)
    xT[:, :0] = 0
    gsl = np.full((CORES, cfg.NCH), 999.0, np.float32)
    for c in range(CORES):
        xT[:, c * cfg.NCH:c * cfg.NCH + NPC] = x[c * NPC:(c + 1) * NPC].T
        gsl[c, :NPC] = batch[c * NPC:(c + 1) * NPC]

    in_maps = []
    for c in range(CORES):
        in_maps.append(dict(
            xT=np.ascontiguousarray(xT[:, c * cfg.NCH:(c + 1) * cfg.NCH]),
            rhs1=rhs1, rhs2=rhs2, fcW=fcW.astype(np.float32),
            iota128=iota128, iota512=iota512, invc=invc_b,
            gslot=gsl[c].reshape(CH, P).T.copy(),
            **edges[c],
        ))
    return in_maps


# -------------------------------------------------------------- bass builder

def build_nc(cfg, stop_after=4):
    N, CH, T, T_LO = cfg.N, cfg.CH, cfg.T, cfg.T_LO
    SEC_LO, SEC_HI, EC, NPC = cfg.SEC_LO, cfg.SEC_HI, cfg.EC, cfg.NPC
    HID, G = cfg.HID, cfg.G
    ROW1 = 192  # [h0(64) 1 h1(64) 1 u u u' u' pad] fp32 -> 768B
    ROW2 = 128  # [h2(64) 1 u u' pad] fp32 -> 512B
    HALF = cfg.HALF
    R = list(range(cfg.CORES))

    nc = bacc.Bacc()
    pi = lambda n, s, d=F32: nc.declare_dram_parameter(n, s, d, isOutput=False)
    xT = pi("xT", [cfg.IN, cfg.NCH])
    rhs1 = pi("rhs1", [cfg.IN, 132])
    rhs2 = pi("rhs2", [2 * HID, HID + 2])
    fcW = pi("fcW", [HID, 2])
    iota128 = pi("iota128", [P, P])
    iota512 = pi("iota512", [P, G])
    invc = pi("invc", [HID, G])
    gslot = pi("gslot", [P, CH])
    gl = pi("gl", [P, CH * SEC_LO // 16], I16)
    gh = pi("gh", [P, CH * SEC_HI // 16], I16)
    vi = pi("vi", [P, CH * EC // 16], I16)
    slot = pi("slot", [P, CH * T])
    out_lg = nc.declare_dram_parameter("out_lg", [G, 2], F32, isOutput=True)

    shard1 = nc.dram_tensor("shard1", [NPC, ROW1], F32)
    table1 = nc.dram_tensor("table1", [N, ROW1], F32, addr_space="Shared")
    vtab1 = nc.dram_tensor("vtab1", [cfg.NCH, 64], F32)
    shard2 = nc.dram_tensor("shard2", [NPC, ROW2], F32)
    table2 = nc.dram_tensor("table2", [N, ROW2], F32, addr_space="Shared")
    vtab2 = nc.dram_tensor("vtab2", [cfg.NCH, 64], F32)
    pool_loc = nc.dram_tensor("pool_loc", [HID, G], F32)
    pool_sh = nc.dram_tensor("pool_sh", [HID, G], F32, addr_space="Shared")

    groups = [tuple(range(a, min(a + cfg.G_CH, CH))) for a in range(0, CH, cfg.G_CH)]

    # SWDGE descriptor-ring pacing: each dma_gather occupies ~num_idxs/16 + 1
    # ring entries until its DMA drains; the ring holds 128 and overrunning it
    # crashes the device. After each gather a 1-element DVE probe-read of its
    # output marks completion; later gathers take a cross-engine dep on the
    # probe so outstanding entries stay under budget.
    gather_fifo = []

    def paced_gather(probe_pool, **kw):
        e = kw["num_idxs"] // 16 + 1
        inst = nc.gpsimd.dma_gather(single_packet=False, **kw)
        gp_t = probe_pool.tile([1, 2], F32, tag="gprobe", name="gprobe")
        rd = nc.vector.tensor_copy(out=gp_t[:], in_=kw["out_ap"][0:1, 0, 0:2])
        tot = sum(x[1] for x in gather_fifo) + e
        while gather_fifo and (tot > 110 or len(gather_fifo) >= 2):
            _, eo, rdo = gather_fifo.pop(0)
            add_dep_helper(inst.ins, rdo.ins, sync=True, reason="swdge ring pacing")
            tot -= eo
        gather_fifo.append((inst, e, rd))
        return inst

    with tile.TileContext(nc) as tc, ExitStack() as ctx:
        cp = ctx.enter_context(tc.tile_pool(name="const", bufs=1))
        dio = ctx.enter_context(tc.tile_pool(name="dio", bufs=3))
        dps = ctx.enter_context(tc.tile_pool(name="dps", bufs=2, space="PSUM"))
        o1p = ctx.enter_context(tc.tile_pool(name="o1p", bufs=1))
        ixp = ctx.enter_context(tc.tile_pool(name="ixp", bufs=2))
        gp = ctx.enter_context(tc.tile_pool(name="gp", bufs=2))
        sxp = ctx.enter_context(tc.tile_pool(name="sxp", bufs=4))
        xp = ctx.enter_context(tc.tile_pool(name="xp", bufs=3))
        ups = ctx.enter_context(tc.tile_pool(name="ups", bufs=2, space="PSUM"))
        pps = ctx.enter_context(tc.tile_pool(name="pps", bufs=1, space="PSUM"))
        fin = ctx.enter_context(tc.tile_pool(name="fin", bufs=3))

        io128 = cp.tile([P, P], F32)
        nc.sync.dma_start(out=io128[:], in_=iota128[:])
        io512 = cp.tile([P, G], F32)
        nc.sync.dma_start(out=io512[:], in_=iota512[:])
        r1sb = cp.tile([cfg.IN, 132], F32)
        nc.sync.dma_start(out=r1sb[:], in_=rhs1[:])
        r2sb = cp.tile([2 * HID, HID + 2], F32)
        nc.sync.dma_start(out=r2sb[:], in_=rhs2[:])
        fcsb = cp.tile([HID, 2], F32)
        nc.sync.dma_start(out=fcsb[:], in_=fcW[:])
        icsb = cp.tile([HID, G], F32)
        nc.sync.dma_start(out=icsb[:], in_=invc[:])
        gssb = cp.tile([P, CH], F32)
        nc.sync.dma_start(out=gssb[:], in_=gslot[:])
        slsb = cp.tile([P, CH * T], F32)
        nc.sync.dma_start(out=slsb[:], in_=slot[:])
        idsb = cp.tile([P, P], F32)
        make_identity(nc, idsb[:])
        out1 = o1p.tile([P, CH * P], F32)

        # ---------------- dense 1: rows of table1 + vtab1 ----------------
        for t in range(CH):
            nv = min(P, NPC - t * P)
            xt = dio.tile([P, P], F32, tag="xt")
            nc.sync.dma_start(out=xt[:], in_=xT[:, t * P:(t + 1) * P])
            ps = dps.tile([P, 132], F32, tag="dtmp")
            nc.tensor.matmul(out=ps[:], lhsT=xt[:], rhs=r1sb[:], start=True, stop=True)
            row = dio.tile([P, ROW1], F32, tag="row1")
            nc.vector.tensor_copy(out=row[:, 0:64], in_=ps[:, 0:64])
            nc.vector.tensor_copy(out=row[:, 65:129], in_=ps[:, 64:128])
            nc.vector.memset(row[:, 64:65], 1.0)
            nc.vector.memset(row[:, 129:130], 1.0)
            nc.scalar.activation(out=row[:, 130:132], in_=ps[:, 128:130], func=AF.Exp, scale=1.0)
            nc.scalar.activation(out=row[:, 132:134], in_=ps[:, 128:130], func=AF.Exp, scale=0.2)
            nc.vector.memset(row[:, 134:192], 0.0)
            vrow = dio.tile([P, 64], F32, tag="vrow")
            nc.scalar.activation(out=vrow[:, 0:2], in_=ps[:, 130:132], func=AF.Exp, scale=1.0)
            nc.scalar.activation(out=vrow[:, 2:4], in_=ps[:, 130:132], func=AF.Exp, scale=0.2)
            nc.vector.memset(vrow[:, 4:64], 0.0)
            nc.sync.dma_start(out=shard1[t * P:t * P + nv, :], in_=row[:nv, :])
            nc.sync.dma_start(out=vtab1[t * P:(t + 1) * P, :], in_=vrow[:])

        tc.strict_bb_all_engine_barrier()
        nc.gpsimd.collective_compute(
            "AllGather", ALU.bypass, replica_groups=[R],
            ins=[shard1[:]], outs=[table1[:]])

        # ---------------- edge phase (shared for both layers) ----------------
        def edge_layer(tabA, tabB, vtab, row_w, nheads, finalize):
            SUB = int(os.environ.get("EDGE_SUB", "4"))
            rw16 = row_w  # elem size in f32 elements
            for grp in groups:
                g0, ng = grp[0], len(grp)
                nlo, nhi, nec = ng * SEC_LO, ng * SEC_HI, ng * EC
                glt = ixp.tile([P, nlo // 16], I16, tag="glt")
                nc.sync.dma_start(out=glt[:], in_=gl[:, g0 * SEC_LO // 16:(g0 * SEC_LO + nlo) // 16])
                ght = ixp.tile([P, nhi // 16], I16, tag="ght")
                nc.sync.dma_start(out=ght[:], in_=gh[:, g0 * SEC_HI // 16:(g0 * SEC_HI + nhi) // 16])
                vit = ixp.tile([P, nec // 16], I16, tag="vit")
                nc.sync.dma_start(out=vit[:], in_=vi[:, g0 * EC // 16:(g0 * EC + nec) // 16])
                hgl = gp.tile([P, nlo // P, rw16], F32, tag="hgl")
                paced_gather(xp, out_ap=hgl[:], in_ap=tabA, idxs_ap=glt[:],
                             num_idxs=nlo, num_idxs_reg=nlo, elem_size=rw16)
                hgh = gp.tile([P, nhi // P, rw16], F32, tag="hgh")
                paced_gather(xp, out_ap=hgh[:], in_ap=tabB, idxs_ap=ght[:],
                             num_idxs=nhi, num_idxs_reg=nhi, elem_size=rw16)
                vg = gp.tile([P, nec // P, 64], F32, tag="vg")
                paced_gather(xp, out_ap=vg[:], in_ap=vtab[:], idxs_ap=vit[:],
                             num_idxs=nec, num_idxs_reg=nec, elem_size=64)
                for ci, c in enumerate(grp):
                    if SUB < 1:
                        continue
                    H2 = 2 * nheads
                    xsb = xp.tile([P, T, 2 * nheads], F32, tag="xsb")
                    m1 = xp.tile([P, T, 2 * nheads], F32, tag="m1")
                    for sec, hg_t, t0, nt in ((0, hgl, 0, T_LO), (1, hgh, T_LO, T - T_LO)):
                        hsl = hg_t[:, ci * nt:(ci + 1) * nt, :]
                        vsl = vg[:, ci * T + t0:ci * T + t0 + nt, :]
                        # u,u' at row cols [64*nheads + nheads + ...]; layout L1: 130..134, L2: 65..67
                        uo = 130 if nheads == 2 else 65
                        nc.vector.tensor_tensor(
                            out=m1[:, t0:t0 + nt, 0:nheads], in0=hsl[:, :, uo:uo + nheads],
                            in1=vsl[:, :, 0:nheads], op=ALU.mult)
                        nc.vector.tensor_tensor(
                            out=xsb[:, t0:t0 + nt, 0:nheads], in0=hsl[:, :, uo + nheads:uo + H2],
                            in1=vsl[:, :, nheads:H2], op=ALU.mult)
                        nc.vector.tensor_tensor(
                            out=xsb[:, t0:t0 + nt, 0:nheads], in0=m1[:, t0:t0 + nt, 0:nheads],
                            in1=xsb[:, t0:t0 + nt, 0:nheads], op=ALU.max)
                    if SUB < 2:
                        continue
                    Us = [ups.tile([P, 65], F32, tag=f"U{h}", name=f"U{h}") for h in range(nheads)]
                    for t in range(T):
                        if t < T_LO:
                            hg_t, tt, nt = hgl, t, T_LO
                        else:
                            hg_t, tt, nt = hgh, t - T_LO, T - T_LO
                        for h in range(nheads):
                            S = sxp.tile([P, P], F32, tag=f"S{h}")
                            nc.vector.tensor_scalar(
                                out=S[:], in0=io128[:],
                                scalar1=slsb[:, c * T + t:c * T + t + 1],
                                scalar2=xsb[:, t, h:h + 1],
                                op0=ALU.is_equal, op1=ALU.mult)
                            if SUB >= 3:
                                nc.tensor.matmul(
                                    out=Us[h][:], lhsT=S[:],
                                    rhs=hg_t[:, ci * nt + tt, h * 65:(h + 1) * 65],
                                    start=(t == 0), stop=(t == T - 1))
                    if SUB >= 4:
                        finalize(c, Us)

        def fin1(c, Us):
            den = fin.tile([P, 2], F32, tag="den1")
            rd = fin.tile([P, 2], F32, tag="rd1")
            for h in range(2):
                nc.vector.tensor_scalar(out=den[:, h:h + 1], in0=Us[h][:, 64:65],
                                        scalar1=1e-20, scalar2=None, op0=ALU.add)
            nc.vector.reciprocal(out=rd[:], in_=den[:])
            for h in range(2):
                nc.vector.tensor_scalar(
                    out=out1[:, c * P + h * 64:c * P + (h + 1) * 64],
                    in0=Us[h][:, 0:64], scalar1=rd[:, h:h + 1], scalar2=0.0,
                    op0=ALU.mult, op1=ALU.max)

        if stop_after >= 2:
            edge_layer(table1[0:HALF, :], table1[HALF:N, :], vtab1, 192, 2, fin1)

        # ---------------- dense 2 ----------------
        for t in range(CH) if stop_after >= 3 else []:
            nv = min(P, NPC - t * P)
            tp = dps.tile([P, P], F32, tag="dtmp")
            nc.tensor.transpose(out=tp[:], in_=out1[:, t * P:(t + 1) * P], identity=idsb[:])
            h1T = dio.tile([P, P], F32, tag="h1T")
            nc.scalar.copy(out=h1T[:], in_=tp[:])
            ps = dps.tile([P, HID + 2], F32, tag="dtmp")
            nc.tensor.matmul(out=ps[:], lhsT=h1T[:], rhs=r2sb[:], start=True, stop=True)
            row = dio.tile([P, ROW2], F32, tag="row2")
            nc.vector.tensor_copy(out=row[:, 0:64], in_=ps[:, 0:64])
            nc.vector.memset(row[:, 64:65], 1.0)
            nc.scalar.activation(out=row[:, 65:66], in_=ps[:, 64:65], func=AF.Exp, scale=1.0)
            nc.scalar.activation(out=row[:, 66:67], in_=ps[:, 64:65], func=AF.Exp, scale=0.2)
            nc.vector.memset(row[:, 67:128], 0.0)
            vrow = dio.tile([P, 64], F32, tag="vrow2")
            nc.scalar.activation(out=vrow[:, 0:1], in_=ps[:, 65:66], func=AF.Exp, scale=1.0)
            nc.scalar.activation(out=vrow[:, 1:2], in_=ps[:, 65:66], func=AF.Exp, scale=0.2)
            nc.vector.memset(vrow[:, 2:64], 0.0)
            nc.sync.dma_start(out=shard2[t * P:t * P + nv, :], in_=row[:nv, :])
            nc.sync.dma_start(out=vtab2[t * P:(t + 1) * P, :], in_=vrow[:])

        if stop_after >= 3:
            tc.strict_bb_all_engine_barrier()
            nc.gpsimd.collective_compute(
                "AllGather", ALU.bypass, replica_groups=[R],
                ins=[shard2[:]], outs=[table2[:]])

        # ---------------- edge layer 2 + pooling ----------------
        plT = pps.tile([HID, G], F32)

        def fin2(c, Us):
            den = fin.tile([P, 1], F32, tag="den2")
            rd = fin.tile([P, 1], F32, tag="rd2")
            nc.vector.tensor_scalar(out=den[:], in0=Us[0][:, 64:65],
                                    scalar1=1e-20, scalar2=None, op0=ALU.add)
            nc.vector.reciprocal(out=rd[:], in_=den[:])
            o2 = fin.tile([P, HID], F32, tag="o2")
            nc.vector.tensor_scalar(out=o2[:], in0=Us[0][:, 0:64],
                                    scalar1=rd[:], scalar2=0.0,
                                    op0=ALU.mult, op1=ALU.max)
            sg = fin.tile([P, G], F32, tag="sg")
            nc.vector.tensor_scalar(out=sg[:], in0=io512[:],
                                    scalar1=gssb[:, c:c + 1], scalar2=None,
                                    op0=ALU.is_equal)
            nc.tensor.matmul(out=plT[:], lhsT=o2[:], rhs=sg[:],
                             start=(c == 0), stop=(c == CH - 1))

        if stop_after >= 4:
            edge_layer(table2[0:HALF, :], table2[HALF:N, :], vtab2, 128, 1, fin2)
        else:
            zz = fin.tile([HID, G], F32, name="zz")
            nc.vector.memset(zz[:], 0.0)
            nc.tensor.matmul(out=plT[:], lhsT=zz[:, 0:P] if HID >= P else zz[:],
                             rhs=zz[:, 0:G], start=True, stop=True) if False else None
            nc.vector.tensor_copy(out=plT[:], in_=zz[:]) if False else None

        plsb = fin.tile([HID, G], F32)
        if stop_after >= 4:
            nc.vector.tensor_copy(out=plsb[:], in_=plT[:])
        else:
            nc.vector.memset(plsb[:], 0.0)
        nc.sync.dma_start(out=pool_loc[:], in_=plsb[:])
        tc.strict_bb_all_engine_barrier()
        nc.gpsimd.collective_compute(
            "AllReduce", ALU.add, replica_groups=[R],
            ins=[pool_loc[:]], outs=[pool_sh[:]])
        plr = fin.tile([HID, G], F32)
        nc.sync.dma_start(out=plr[:], in_=pool_sh[:])
        nc.vector.tensor_tensor(out=plr[:], in0=plr[:], in1=icsb[:], op=ALU.mult)
        for gt in range(max(1, G // P)):
            gw = min(P, G - gt * P)
            lg = dps.tile([P, 2], F32, tag="dtmp")
            nc.tensor.matmul(out=lg[:gw], lhsT=plr[:, gt * P:gt * P + gw], rhs=fcsb[:],
                             start=True, stop=True)
            mx = fin.tile([P, 1], F32, tag="mx")
            nc.vector.tensor_reduce(out=mx[:gw], in_=lg[:gw], op=ALU.max,
                                    axis=mybir.AxisListType.X)
            t1 = fin.tile([P, 2], F32, tag="t1")
            nc.vector.tensor_scalar(out=t1[:gw], in0=lg[:gw], scalar1=mx[:gw],
                                    scalar2=None, op0=ALU.subtract)
            ex = fin.tile([P, 2], F32, tag="ex")
            es = fin.tile([P, 1], F32, tag="es")
            nc.scalar.activation(out=ex[:gw], in_=t1[:gw], func=AF.Exp, accum_out=es[:gw])
            ln = fin.tile([P, 1], F32, tag="ln")
            nc.scalar.activation(out=ln[:gw], in_=es[:gw], func=AF.Ln)
            lsm = fin.tile([P, 2], F32, tag="lsm")
            nc.vector.tensor_scalar(out=lsm[:gw], in0=t1[:gw], scalar1=ln[:gw],
                                    scalar2=None, op0=ALU.subtract)
            nc.sync.dma_start(out=out_lg[gt * P:gt * P + gw, :], in_=lsm[:gw])

    nc.compile()
    return nc


# ------------------------------------------------------------------ entry

LAST_EXEC_NS = None

def kernel(x, edge_index, batch, W1, a_src1, a_dst1, b1, W2, a_src2, a_dst2, b2,
           fcW, fcb):
    x = np.asarray(x, np.float32)
    edge_index = np.asarray(edge_index, np.int64)
    batch = np.asarray(batch, np.int64)
    for b in (b1, b2, fcb):
        assert np.abs(np.asarray(b)).max() == 0.0, "nonzero bias unsupported"
    cfg = plan_cfg(N=x.shape[0], E0=edge_index.shape[1], G=512)
    in_maps = prep_inputs(cfg, x, edge_index, batch,
                          np.asarray(W1, np.float32), np.asarray(a_src1, np.float32),
                          np.asarray(a_dst1, np.float32), np.asarray(W2, np.float32),
                          np.asarray(a_src2, np.float32), np.asarray(a_dst2, np.float32),
                          np.asarray(fcW, np.float32))
    nc = build_nc(cfg)
    trace = os.environ.get("KERNEL_TRACE") == "1"
    res = run_bass_kernel_spmd(nc, in_maps, list(range(cfg.CORES)), trace=trace)
    global LAST_EXEC_NS
    LAST_EXEC_NS = res.exec_time_ns
    if trace:
        print(f"HW exec time: {res.exec_time_ns} ns "
              f"(mean {res.mean_exec_time_ns} ns)")
    return np.asarray(res.results[0]["out_lg"], np.float32)



# revision 7
# speedup vs baseline: 1.6700x; 1.6700x over previous
"""GAT classifier on 8 trn2 NeuronCores (Bass/Tile) — v2.

Sharding: 1D node partition (6250 nodes/core); edges assigned to the core
owning their dst node, sorted by dst into 128-node chunks.

v2 design (vs v1): the per-edge SWDGE gather count is halved and payloads
move to bf16. Per edge per layer there is exactly ONE gathered element:
  L1: 512B bf16 row [h0(64)|1|h1(64)|1|as0|as1|pad]
  L2: 256B bf16 row [h(64)|1|as|pad]
The dst-side attention term ad_e is produced without any gather: a host
-streamed fp8 one-hot S0T[j,e] (slot-of-edge) is matmul'd against the
local per-chunk ad table (PE, 2 cols), giving per-edge ad in PSUM.
Coefficients are computed directly: coef = exp(max(s, 0.2*s)), s = as+ad.
The segmented softmax + aggregation stays as PSUM-accumulated selection
matmuls with bf16 S matrices; denominators ride the baked-in "1" columns.
Layer tables are AllGathered between layers; pooling uses an AllReduce.
"""
import math
import os
import sys
from contextlib import ExitStack
from dataclasses import dataclass

import numpy as np

for _p in ("/opt/trn_rl_repo", "/root/.axon_site/_ro/trn_rl_repo"):
    if os.path.isdir(_p) and _p not in sys.path:
        sys.path.insert(0, _p)

import concourse.bacc as bacc
import concourse.bass as bass
import concourse.mybir as mybir
import concourse.tile as tile
from concourse.tile import add_dep_helper
from concourse.bass_utils import run_bass_kernel_spmd
from concourse.masks import make_identity

P = 128
AF = mybir.ActivationFunctionType
ALU = mybir.AluOpType
F32 = mybir.dt.float32
BF16 = mybir.dt.bfloat16
FP8 = mybir.dt.float8e4
I16 = mybir.dt.int16
NP_BF16 = mybir.dt.np(BF16)
NP_FP8 = mybir.dt.np(FP8)

ROW1 = 256   # L1 table row: bf16 slots [h0|1|h1|1|as0|as1|pad] -> 512B
ROW2 = 128   # L2 table row: bf16 slots [h|1|as|pad] -> 256B


@dataclass
class Cfg:
    N: int = 50000
    E0: int = 800000
    IN: int = 128
    HID: int = 64
    G: int = 512
    CORES: int = 8
    NPC: int = 0
    CH: int = 0
    HALF: int = 0
    SEC_LO: int = 0
    SEC_HI: int = 0
    EC: int = 0
    T: int = 0
    T_LO: int = 0
    G_CH: int = 2

    @property
    def NCH(self):  # padded per-core node count
        return self.CH * P


def plan_cfg(N, E0, G, CORES=8):
    c = Cfg(N=N, E0=E0, G=G, CORES=CORES)
    assert N % CORES == 0
    c.NPC = N // CORES
    c.CH = math.ceil(c.NPC / P)
    c.HALF = ((N // 2) + 127) & ~127
    assert c.HALF < 32768 * 2 and (N - c.HALF) <= 32767 and c.HALF <= 32767
    return c


# ----------------------------------------------------------------- host prep

def prep_edges(cfg, src, dst):
    """Per-core edge arrays. Returns list of dicts + fills cfg.SEC_*/EC/T."""
    owner = dst // cfg.NPC
    per_core = []
    maxlo = maxhi = 0
    for c in range(cfg.CORES):
        m = owner == c
        s = src[m]
        dl = dst[m] - c * cfg.NPC
        chunk = dl >> 7
        half = (s >= cfg.HALF).astype(np.int64)
        order = np.lexsort((s, half, chunk))
        s, dl, chunk, half = s[order], dl[order], chunk[order], half[order]
        key = chunk * 2 + half
        cnt = np.bincount(key, minlength=cfg.CH * 2).reshape(cfg.CH, 2)
        maxlo = max(maxlo, int(cnt[:, 0].max()))
        maxhi = max(maxhi, int(cnt[:, 1].max()))
        per_core.append((s, dl, chunk, half, cnt))
    cfg.SEC_LO = ((maxlo + 127) & ~127) or P
    cfg.SEC_HI = ((maxhi + 127) & ~127) or P
    cfg.EC = cfg.SEC_LO + cfg.SEC_HI
    cfg.T = cfg.EC // P
    cfg.T_LO = cfg.SEC_LO // P

    out = []
    for c in range(cfg.CORES):
        s, dl, chunk, half, cnt = per_core[c]
        gl = np.zeros((cfg.CH, cfg.SEC_LO), np.int16)
        gh = np.zeros((cfg.CH, cfg.SEC_HI), np.int16)
        sl = np.full((cfg.CH, cfg.EC), 300.0, np.float32)
        ofs = np.zeros(cfg.CH * 2 + 1, np.int64)
        np.cumsum(cnt.reshape(-1), out=ofs[1:])
        for k in range(cfg.CH):
            nlo, nhi = int(cnt[k, 0]), int(cnt[k, 1])
            a = ofs[2 * k]
            gl[k, :nlo] = s[a:a + nlo]
            sl[k, :nlo] = (dl[a:a + nlo] & 127).astype(np.float32)
            b = ofs[2 * k + 1]
            gh[k, :nhi] = s[b:b + nhi] - cfg.HALF
            sl[k, cfg.SEC_LO:cfg.SEC_LO + nhi] = (dl[b:b + nhi] & 127).astype(np.float32)

        def wrap16(a):  # idx i -> [i % 16, i // 16], replicated over 8 groups
            w = a.reshape(-1, 16).T.copy()
            return np.tile(w, (8, 1)).astype(np.int16)

        # S0T fp8 one-hot blocks: [128 j, CH*T*128] — col (c*T+t)*128+e is
        # one at row slot_e (pad slots 300 -> all-zero column).
        sl_t = sl.reshape(cfg.CH * cfg.T, P)  # [tile, e] slot values
        idx = sl_t.astype(np.int32)
        s0t = np.zeros((cfg.CH * cfg.T, P, P), NP_FP8)  # [tile, e, j]
        tt, ee = np.nonzero(idx < P)
        s0t[tt, ee, idx[tt, ee]] = 1.0
        s0t = np.ascontiguousarray(s0t.transpose(2, 0, 1).reshape(P, cfg.CH * cfg.T * P))

        out.append(dict(
            gl=wrap16(gl), gh=wrap16(gh),
            slot=sl.reshape(cfg.CH * cfg.T, P).T.copy(),
            s0t=s0t,
        ))
    return out


def prep_inputs(cfg, x, edge_index, batch, W1, a_src1, a_dst1, W2, a_src2, a_dst2, fcW):
    N, CORES, NPC, CH = cfg.N, cfg.CORES, cfg.NPC, cfg.CH
    src = np.concatenate([edge_index[0], np.arange(N)]).astype(np.int64)
    dst = np.concatenate([edge_index[1], np.arange(N)]).astype(np.int64)
    edges = prep_edges(cfg, src, dst)

    H = 2
    HID = cfg.HID
    rhs1 = np.zeros((cfg.IN, H * HID + 4), np.float32)
    rhs1[:, :H * HID] = W1
    for h in range(H):
        rhs1[:, H * HID + h] = W1[:, h * HID:(h + 1) * HID] @ a_src1[h]
        rhs1[:, H * HID + 2 + h] = W1[:, h * HID:(h + 1) * HID] @ a_dst1[h]
    rhs2 = np.zeros((H * HID, HID + 2), np.float32)
    rhs2[:, :HID] = W2
    rhs2[:, HID] = W2 @ a_src2[0]
    rhs2[:, HID + 1] = W2 @ a_dst2[0]

    iota128 = np.tile(np.arange(P, dtype=np.float32), (P, 1)).astype(NP_BF16)
    iota512 = np.tile(np.arange(cfg.G, dtype=np.float32), (P, 1))
    cnt = np.bincount(batch, minlength=cfg.G).astype(np.float32)
    invc = 1.0 / np.maximum(cnt, 1.0)
    invc_b = np.tile(invc, (HID, 1)).astype(np.float32)

    xT = np.zeros((cfg.IN, CORES * cfg.NCH), np.float32)
    gsl = np.full((CORES, cfg.NCH), 999.0, np.float32)
    for c in range(CORES):
        xT[:, c * cfg.NCH:c * cfg.NCH + NPC] = x[c * NPC:(c + 1) * NPC].T
        gsl[c, :NPC] = batch[c * NPC:(c + 1) * NPC]

    in_maps = []
    for c in range(CORES):
        in_maps.append(dict(
            xT=np.ascontiguousarray(xT[:, c * cfg.NCH:(c + 1) * cfg.NCH]),
            rhs1=rhs1, rhs2=rhs2, fcW=fcW.astype(np.float32),
            iota128=iota128, iota512=iota512, invc=invc_b,
            gslot=gsl[c].reshape(CH, P).T.copy(),
            **edges[c],
        ))
    return in_maps


# -------------------------------------------------------------- bass builder

def build_nc(cfg):
    N, CH, T, T_LO = cfg.N, cfg.CH, cfg.T, cfg.T_LO
    SEC_LO, SEC_HI, EC, NPC = cfg.SEC_LO, cfg.SEC_HI, cfg.EC, cfg.NPC
    HID, G = cfg.HID, cfg.G
    HALF = cfg.HALF
    R = list(range(cfg.CORES))

    nc = bacc.Bacc()
    pi = lambda n, s, d=F32: nc.declare_dram_parameter(n, s, d, isOutput=False)
    xT = pi("xT", [cfg.IN, cfg.NCH])
    rhs1 = pi("rhs1", [cfg.IN, 132])
    rhs2 = pi("rhs2", [2 * HID, HID + 2])
    fcW = pi("fcW", [HID, 2])
    iota128 = pi("iota128", [P, P], BF16)
    iota512 = pi("iota512", [P, G])
    invc = pi("invc", [HID, G])
    gslot = pi("gslot", [P, CH])
    gl = pi("gl", [P, CH * SEC_LO // 16], I16)
    gh = pi("gh", [P, CH * SEC_HI // 16], I16)
    slot = pi("slot", [P, CH * T])
    s0t = pi("s0t", [P, CH * T * P], FP8)
    out_lg = nc.declare_dram_parameter("out_lg", [G, 2], F32, isOutput=True)

    shard1 = nc.dram_tensor("shard1", [NPC, ROW1], BF16)
    table1 = nc.dram_tensor("table1", [N, ROW1], BF16, addr_space="Shared")
    shard2 = nc.dram_tensor("shard2", [NPC, ROW2], BF16)
    table2 = nc.dram_tensor("table2", [N, ROW2], BF16, addr_space="Shared")
    pool_loc = nc.dram_tensor("pool_loc", [HID, G], F32)
    pool_sh = nc.dram_tensor("pool_sh", [HID, G], F32, addr_space="Shared")

    groups = [tuple(range(a, min(a + cfg.G_CH, CH))) for a in range(0, CH, cfg.G_CH)]

    # SWDGE descriptor-ring pacing (see v1): keep outstanding entries under
    # budget via probe-read cross-engine deps.
    gather_fifo = []

    def paced_gather(probe_pool, **kw):
        e = kw["num_idxs"] // 16 + 1
        inst = nc.gpsimd.dma_gather(single_packet=False, **kw)
        gp_t = probe_pool.tile([1, 2], BF16, tag="gprobe", name="gprobe")
        rd = nc.vector.tensor_copy(out=gp_t[:], in_=kw["out_ap"][0:1, 0, 0:2])
        tot = sum(x[1] for x in gather_fifo) + e
        while gather_fifo and (tot > 110 or len(gather_fifo) >= 2):
            _, eo, rdo = gather_fifo.pop(0)
            add_dep_helper(inst.ins, rdo.ins, sync=True, reason="swdge ring pacing")
            tot -= eo
        gather_fifo.append((inst, e, rd))
        return inst

    with tile.TileContext(nc) as tc, ExitStack() as ctx:
        cp = ctx.enter_context(tc.tile_pool(name="const", bufs=1))
        dio = ctx.enter_context(tc.tile_pool(name="dio", bufs=3))
        dps = ctx.enter_context(tc.tile_pool(name="dps", bufs=2, space="PSUM"))
        o1p = ctx.enter_context(tc.tile_pool(name="o1p", bufs=1))
        ixp = ctx.enter_context(tc.tile_pool(name="ixp", bufs=2))
        gp = ctx.enter_context(tc.tile_pool(name="gp", bufs=2))
        stp = ctx.enter_context(tc.tile_pool(name="stp", bufs=2))
        sxp = ctx.enter_context(tc.tile_pool(name="sxp", bufs=4))
        xp = ctx.enter_context(tc.tile_pool(name="xp", bufs=3))
        ups = ctx.enter_context(tc.tile_pool(name="ups", bufs=2, space="PSUM"))
        aps = ctx.enter_context(tc.tile_pool(name="aps", bufs=1, space="PSUM"))
        pps = ctx.enter_context(tc.tile_pool(name="pps", bufs=1, space="PSUM"))
        fin = ctx.enter_context(tc.tile_pool(name="fin", bufs=3))

        io128 = cp.tile([P, P], BF16)
        nc.sync.dma_start(out=io128[:], in_=iota128[:])
        io512 = cp.tile([P, G], F32)
        nc.sync.dma_start(out=io512[:], in_=iota512[:])
        r1sb = cp.tile([cfg.IN, 132], F32)
        nc.sync.dma_start(out=r1sb[:], in_=rhs1[:])
        r2sb = cp.tile([2 * HID, HID + 2], F32)
        nc.sync.dma_start(out=r2sb[:], in_=rhs2[:])
        fcsb = cp.tile([HID, 2], F32)
        nc.sync.dma_start(out=fcsb[:], in_=fcW[:])
        icsb = cp.tile([HID, G], F32)
        nc.sync.dma_start(out=icsb[:], in_=invc[:])
        gssb = cp.tile([P, CH], F32)
        nc.sync.dma_start(out=gssb[:], in_=gslot[:])
        slsb = cp.tile([P, CH * T], F32)
        nc.sync.dma_start(out=slsb[:], in_=slot[:])
        idsb = cp.tile([P, P], F32)
        make_identity(nc, idsb[:])
        out1 = o1p.tile([P, CH * P], F32)
        adloc1 = cp.tile([P, CH, 2], BF16)  # per-chunk dst attention terms
        adloc2 = cp.tile([P, CH, 1], BF16)

        # ---------------- dense 1: rows of table1 ----------------
        for t in range(CH):
            nv = min(P, NPC - t * P)
            xt = dio.tile([P, P], F32, tag="xt")
            nc.sync.dma_start(out=xt[:], in_=xT[:, t * P:(t + 1) * P])
            ps = dps.tile([P, 132], F32, tag="dtmp")
            nc.tensor.matmul(out=ps[:], lhsT=xt[:], rhs=r1sb[:], start=True, stop=True)
            row = dio.tile([P, ROW1], BF16, tag="row1")
            nc.vector.tensor_copy(out=row[:, 0:64], in_=ps[:, 0:64])
            nc.vector.tensor_copy(out=row[:, 65:129], in_=ps[:, 64:128])
            nc.vector.memset(row[:, 64:65], 1.0)
            nc.vector.memset(row[:, 129:130], 1.0)
            nc.vector.tensor_copy(out=row[:, 130:132], in_=ps[:, 128:130])
            nc.vector.memset(row[:, 132:ROW1], 0.0)
            nc.vector.tensor_copy(out=adloc1[:, t, :], in_=ps[:, 130:132])
            nc.sync.dma_start(out=shard1[t * P:t * P + nv, :], in_=row[:nv, :])

        tc.strict_bb_all_engine_barrier()
        nc.gpsimd.collective_compute(
            "AllGather", ALU.bypass, replica_groups=[R],
            ins=[shard1[:]], outs=[table1[:]])

        # ---------------- edge phase (both layers) ----------------
        def edge_layer(tabA, tabB, adloc, row_w, nheads, finalize):
            for grp in groups:
                g0, ng = grp[0], len(grp)
                nlo, nhi = ng * SEC_LO, ng * SEC_HI
                glt = ixp.tile([P, nlo // 16], I16, tag="glt")
                nc.sync.dma_start(out=glt[:], in_=gl[:, g0 * SEC_LO // 16:(g0 * SEC_LO + nlo) // 16])
                ght = ixp.tile([P, nhi // 16], I16, tag="ght")
                nc.sync.dma_start(out=ght[:], in_=gh[:, g0 * SEC_HI // 16:(g0 * SEC_HI + nhi) // 16])
                stt = stp.tile([P, ng * T * P], FP8, tag="stt")
                nc.sync.dma_start(out=stt[:], in_=s0t[:, g0 * T * P:(g0 + ng) * T * P])
                hgl = gp.tile([P, nlo // P, row_w], BF16, tag="hgl")
                paced_gather(xp, out_ap=hgl[:], in_ap=tabA, idxs_ap=glt[:],
                             num_idxs=nlo, num_idxs_reg=nlo, elem_size=row_w)
                hgh = gp.tile([P, nhi // P, row_w], BF16, tag="hgh")
                paced_gather(xp, out_ap=hgh[:], in_ap=tabB, idxs_ap=ght[:],
                             num_idxs=nhi, num_idxs_reg=nhi, elem_size=row_w)
                for ci, c in enumerate(grp):
                    # per-edge ad via fp8 one-hot matmul against local ad chunk
                    adps = aps.tile([P, T, nheads], F32, tag="adps")
                    for t in range(T):
                        nc.tensor.matmul(
                            out=adps[:, t, :],
                            lhsT=stt[:, (ci * T + t) * P:(ci * T + t + 1) * P],
                            rhs=adloc[:, c, :], start=True, stop=True)
                    adsb = xp.tile([P, T, nheads], BF16, tag="adsb")
                    nc.vector.tensor_copy(out=adsb[:], in_=adps[:])
                    # s = as + ad ; x = exp(max(s, 0.2 s))
                    ssb = xp.tile([P, T, nheads], BF16, tag="ssb")
                    uo = 130 if nheads == 2 else 65
                    for sec, hg_t, t0, nt in ((0, hgl, 0, T_LO), (1, hgh, T_LO, T - T_LO)):
                        nc.vector.tensor_tensor(
                            out=ssb[:, t0:t0 + nt, :],
                            in0=hg_t[:, ci * nt:(ci + 1) * nt, uo:uo + nheads],
                            in1=adsb[:, t0:t0 + nt, :], op=ALU.add)
                    s2 = xp.tile([P, T, nheads], BF16, tag="s2")
                    nc.vector.tensor_scalar(out=s2[:], in0=ssb[:], scalar1=0.2,
                                            scalar2=None, op0=ALU.mult)
                    nc.vector.tensor_tensor(out=s2[:], in0=ssb[:], in1=s2[:], op=ALU.max)
                    xsb = xp.tile([P, T, nheads], F32, tag="xsb")
                    nc.scalar.activation(out=xsb[:], in_=s2[:], func=AF.Exp, scale=1.0)
                    # selection matmuls
                    Us = [ups.tile([P, 65], F32, tag=f"U{h}", name=f"U{h}") for h in range(nheads)]
                    for t in range(T):
                        if t < T_LO:
                            hg_t, tt, nt = hgl, t, T_LO
                        else:
                            hg_t, tt, nt = hgh, t - T_LO, T - T_LO
                        for h in range(nheads):
                            S = sxp.tile([P, P], BF16, tag=f"S{h}")
                            nc.vector.tensor_scalar(
                                out=S[:], in0=io128[:],
                                scalar1=slsb[:, c * T + t:c * T + t + 1],
                                scalar2=xsb[:, t, h:h + 1],
                                op0=ALU.is_equal, op1=ALU.mult)
                            nc.tensor.matmul(
                                out=Us[h][:], lhsT=S[:],
                                rhs=hg_t[:, ci * nt + tt, h * 65:(h + 1) * 65],
                                start=(t == 0), stop=(t == T - 1))
                    finalize(c, Us)

        def fin1(c, Us):
            den = fin.tile([P, 2], F32, tag="den1")
            rd = fin.tile([P, 2], F32, tag="rd1")
            for h in range(2):
                nc.vector.tensor_scalar(out=den[:, h:h + 1], in0=Us[h][:, 64:65],
                                        scalar1=1e-20, scalar2=None, op0=ALU.add)
            nc.vector.reciprocal(out=rd[:], in_=den[:])
            for h in range(2):
                nc.vector.tensor_scalar(
                    out=out1[:, c * P + h * 64:c * P + (h + 1) * 64],
                    in0=Us[h][:, 0:64], scalar1=rd[:, h:h + 1], scalar2=0.0,
                    op0=ALU.mult, op1=ALU.max)

        edge_layer(table1[0:HALF, :], table1[HALF:N, :], adloc1, ROW1, 2, fin1)

        # ---------------- dense 2 ----------------
        for t in range(CH):
            nv = min(P, NPC - t * P)
            tp = dps.tile([P, P], F32, tag="dtmp")
            nc.tensor.transpose(out=tp[:], in_=out1[:, t * P:(t + 1) * P], identity=idsb[:])
            h1T = dio.tile([P, P], F32, tag="h1T")
            nc.scalar.copy(out=h1T[:], in_=tp[:])
            ps = dps.tile([P, HID + 2], F32, tag="dtmp")
            nc.tensor.matmul(out=ps[:], lhsT=h1T[:], rhs=r2sb[:], start=True, stop=True)
            row = dio.tile([P, ROW2], BF16, tag="row2")
            nc.vector.tensor_copy(out=row[:, 0:64], in_=ps[:, 0:64])
            nc.vector.memset(row[:, 64:65], 1.0)
            nc.vector.tensor_copy(out=row[:, 65:66], in_=ps[:, 64:65])
            nc.vector.memset(row[:, 66:ROW2], 0.0)
            nc.vector.tensor_copy(out=adloc2[:, t, :], in_=ps[:, 65:66])
            nc.sync.dma_start(out=shard2[t * P:t * P + nv, :], in_=row[:nv, :])

        tc.strict_bb_all_engine_barrier()
        nc.gpsimd.collective_compute(
            "AllGather", ALU.bypass, replica_groups=[R],
            ins=[shard2[:]], outs=[table2[:]])

        # ---------------- edge layer 2 + pooling ----------------
        plT = pps.tile([HID, G], F32, name="plT")

        def fin2(c, Us):
            den = fin.tile([P, 1], F32, tag="den2")
            rd = fin.tile([P, 1], F32, tag="rd2")
            nc.vector.tensor_scalar(out=den[:], in0=Us[0][:, 64:65],
                                    scalar1=1e-20, scalar2=None, op0=ALU.add)
            nc.vector.reciprocal(out=rd[:], in_=den[:])
            o2 = fin.tile([P, HID], F32, tag="o2")
            nc.vector.tensor_scalar(out=o2[:], in0=Us[0][:, 0:64],
                                    scalar1=rd[:], scalar2=0.0,
                                    op0=ALU.mult, op1=ALU.max)
            sg = fin.tile([P, G], F32, tag="sg")
            nc.vector.tensor_scalar(out=sg[:], in0=io512[:],
                                    scalar1=gssb[:, c:c + 1], scalar2=None,
                                    op0=ALU.is_equal)
            nc.tensor.matmul(out=plT[:], lhsT=o2[:], rhs=sg[:],
                             start=(c == 0), stop=(c == CH - 1))

        edge_layer(table2[0:HALF, :], table2[HALF:N, :], adloc2, ROW2, 1, fin2)

        plsb = fin.tile([HID, G], F32)
        nc.vector.tensor_copy(out=plsb[:], in_=plT[:])
        nc.sync.dma_start(out=pool_loc[:], in_=plsb[:])
        tc.strict_bb_all_engine_barrier()
        nc.gpsimd.collective_compute(
            "AllReduce", ALU.add, replica_groups=[R],
            ins=[pool_loc[:]], outs=[pool_sh[:]])
        plr = fin.tile([HID, G], F32)
        nc.sync.dma_start(out=plr[:], in_=pool_sh[:])
        nc.vector.tensor_tensor(out=plr[:], in0=plr[:], in1=icsb[:], op=ALU.mult)
        for gt in range(max(1, G // P)):
            gw = min(P, G - gt * P)
            lg = dps.tile([P, 2], F32, tag="dtmp")
            nc.tensor.matmul(out=lg[:gw], lhsT=plr[:, gt * P:gt * P + gw], rhs=fcsb[:],
                             start=True, stop=True)
            mx = fin.tile([P, 1], F32, tag="mx")
            nc.vector.tensor_reduce(out=mx[:gw], in_=lg[:gw], op=ALU.max,
                                    axis=mybir.AxisListType.X)
            t1 = fin.tile([P, 2], F32, tag="t1")
            nc.vector.tensor_scalar(out=t1[:gw], in0=lg[:gw], scalar1=mx[:gw],
                                    scalar2=None, op0=ALU.subtract)
            ex = fin.tile([P, 2], F32, tag="ex")
            es = fin.tile([P, 1], F32, tag="es")
            nc.scalar.activation(out=ex[:gw], in_=t1[:gw], func=AF.Exp, accum_out=es[:gw])
            ln = fin.tile([P, 1], F32, tag="ln")
            nc.scalar.activation(out=ln[:gw], in_=es[:gw], func=AF.Ln)
            lsm = fin.tile([P, 2], F32, tag="lsm")
            nc.vector.tensor_scalar(out=lsm[:gw], in0=t1[:gw], scalar1=ln[:gw],
                                    scalar2=None, op0=ALU.subtract)
            nc.sync.dma_start(out=out_lg[gt * P:gt * P + gw, :], in_=lsm[:gw])

    nc.compile()
    return nc


# ------------------------------------------------------------------ entry

LAST_EXEC_NS = None

def kernel(x, edge_index, batch, W1, a_src1, a_dst1, b1, W2, a_src2, a_dst2, b2,
           fcW, fcb):
    x = np.asarray(x, np.float32)
    edge_index = np.asarray(edge_index, np.int64)
    batch = np.asarray(batch, np.int64)
    for b in (b1, b2, fcb):
        assert np.abs(np.asarray(b)).max() == 0.0, "nonzero bias unsupported"
    cfg = plan_cfg(N=x.shape[0], E0=edge_index.shape[1], G=512)
    in_maps = prep_inputs(cfg, x, edge_index, batch,
                          np.asarray(W1, np.float32), np.asarray(a_src1, np.float32),
                          np.asarray(a_dst1, np.float32), np.asarray(W2, np.float32),
                          np.asarray(a_src2, np.float32), np.asarray(a_dst2, np.float32),
                          np.asarray(fcW, np.float32))
    nc = build_nc(cfg)
    trace = os.environ.get("KERNEL_TRACE") == "1"
    res = run_bass_kernel_spmd(nc, in_maps, list(range(cfg.CORES)), trace=trace)
    global LAST_EXEC_NS
    LAST_EXEC_NS = res.exec_time_ns
    if trace:
        print(f"HW exec time: {res.exec_time_ns} ns "
              f"(mean {res.mean_exec_time_ns} ns)")
    return np.asarray(res.results[0]["out_lg"], np.float32)


# revision 9
# speedup vs baseline: 2.5576x; 1.5315x over previous
"""GAT classifier on 8 trn2 NeuronCores (Bass/Tile) — v2.

Sharding: 1D node partition (6250 nodes/core); edges assigned to the core
owning their dst node, sorted by dst into 128-node chunks.

v2 design (vs v1): the per-edge SWDGE gather count is halved and payloads
move to bf16. Per edge per layer there is exactly ONE gathered element:
  L1: 512B bf16 row [h0(64)|1|h1(64)|1|as0|as1|pad]
  L2: 256B bf16 row [h(64)|1|as|pad]
The dst-side attention term ad_e is produced without any gather: a host
-streamed fp8 one-hot S0T[j,e] (slot-of-edge) is matmul'd against the
local per-chunk ad table (PE, 2 cols), giving per-edge ad in PSUM.
Coefficients are computed directly: coef = exp(max(s, 0.2*s)), s = as+ad.
The segmented softmax + aggregation stays as PSUM-accumulated selection
matmuls with bf16 S matrices; denominators ride the baked-in "1" columns.
Layer tables are AllGathered between layers; pooling uses an AllReduce.
"""
import math
import os
import sys
from contextlib import ExitStack
from dataclasses import dataclass

import numpy as np

for _p in ("/opt/trn_rl_repo", "/root/.axon_site/_ro/trn_rl_repo"):
    if os.path.isdir(_p) and _p not in sys.path:
        sys.path.insert(0, _p)

import concourse.bacc as bacc
import concourse.bass as bass
import concourse.mybir as mybir
import concourse.tile as tile
from concourse.tile import add_dep_helper
from concourse.bass_utils import run_bass_kernel_spmd
from concourse.masks import make_identity

P = 128
AF = mybir.ActivationFunctionType
ALU = mybir.AluOpType
F32 = mybir.dt.float32
BF16 = mybir.dt.bfloat16
FP8 = mybir.dt.float8e4
I16 = mybir.dt.int16
NP_BF16 = mybir.dt.np(BF16)
NP_FP8 = mybir.dt.np(FP8)

ROW1 = 256   # L1 table row: bf16 slots [h0|1|h1|1|as0|as1|pad] -> 512B
ROW2 = 128   # L2 table row: bf16 slots [h|1|as|pad] -> 256B


@dataclass
class Cfg:
    N: int = 50000
    E0: int = 800000
    IN: int = 128
    HID: int = 64
    G: int = 512
    CORES: int = 8
    NPC: int = 0
    CH: int = 0
    HALF: int = 0
    SEC_LO: int = 0
    SEC_HI: int = 0
    EC: int = 0
    T: int = 0
    T_LO: int = 0
    G_CH: int = 2

    @property
    def NCH(self):  # padded per-core node count
        return self.CH * P


def plan_cfg(N, E0, G, CORES=8):
    c = Cfg(N=N, E0=E0, G=G, CORES=CORES)
    assert N % CORES == 0
    c.NPC = N // CORES
    c.CH = math.ceil(c.NPC / P)
    c.HALF = ((N // 2) + 127) & ~127
    assert c.HALF < 32768 * 2 and (N - c.HALF) <= 32767 and c.HALF <= 32767
    return c


# ----------------------------------------------------------------- host prep

def prep_edges(cfg, src, dst):
    """Per-core edge arrays. Returns list of dicts + fills cfg.SEC_*/EC/T."""
    owner = dst // cfg.NPC
    per_core = []
    maxlo = maxhi = 0
    for c in range(cfg.CORES):
        m = owner == c
        s = src[m]
        dl = dst[m] - c * cfg.NPC
        chunk = dl >> 7
        half = (s >= cfg.HALF).astype(np.int64)
        order = np.lexsort((s, half, chunk))
        s, dl, chunk, half = s[order], dl[order], chunk[order], half[order]
        key = chunk * 2 + half
        cnt = np.bincount(key, minlength=cfg.CH * 2).reshape(cfg.CH, 2)
        maxlo = max(maxlo, int(cnt[:, 0].max()))
        maxhi = max(maxhi, int(cnt[:, 1].max()))
        per_core.append((s, dl, chunk, half, cnt))
    cfg.SEC_LO = ((maxlo + 127) & ~127) or P
    cfg.SEC_HI = ((maxhi + 127) & ~127) or P
    cfg.EC = cfg.SEC_LO + cfg.SEC_HI
    cfg.T = cfg.EC // P
    cfg.T_LO = cfg.SEC_LO // P

    out = []
    for c in range(cfg.CORES):
        s, dl, chunk, half, cnt = per_core[c]
        gl = np.zeros((cfg.CH, cfg.SEC_LO), np.int16)
        gh = np.zeros((cfg.CH, cfg.SEC_HI), np.int16)
        sl = np.full((cfg.CH, cfg.EC), 300.0, np.float32)
        ofs = np.zeros(cfg.CH * 2 + 1, np.int64)
        np.cumsum(cnt.reshape(-1), out=ofs[1:])
        for k in range(cfg.CH):
            nlo, nhi = int(cnt[k, 0]), int(cnt[k, 1])
            a = ofs[2 * k]
            gl[k, :nlo] = s[a:a + nlo]
            sl[k, :nlo] = (dl[a:a + nlo] & 127).astype(np.float32)
            b = ofs[2 * k + 1]
            gh[k, :nhi] = s[b:b + nhi] - cfg.HALF
            sl[k, cfg.SEC_LO:cfg.SEC_LO + nhi] = (dl[b:b + nhi] & 127).astype(np.float32)

        def wrap16(a):  # idx i -> [i % 16, i // 16], replicated over 8 groups
            w = a.reshape(-1, 16).T.copy()
            return np.tile(w, (8, 1)).astype(np.int16)

        # S0T fp8 one-hot blocks: [128 j, CH*T*128] — col (c*T+t)*128+e is
        # one at row slot_e (pad slots 300 -> all-zero column).
        sl_t = sl.reshape(cfg.CH * cfg.T, P)  # [tile, e] slot values
        idx = sl_t.astype(np.int32)
        s0t = np.zeros((cfg.CH * cfg.T, P, P), NP_FP8)  # [tile, e, j]
        tt, ee = np.nonzero(idx < P)
        s0t[tt, ee, idx[tt, ee]] = 1.0
        s0t = np.ascontiguousarray(s0t.transpose(2, 0, 1).reshape(P, cfg.CH * cfg.T * P))

        out.append(dict(
            gl=wrap16(gl), gh=wrap16(gh),
            slot=sl.reshape(cfg.CH * cfg.T, P).T.copy(),
            s0t=s0t,
        ))
    return out


def balance_perm(cfg, dst):
    """Per-core node->slot permutation equalizing per-chunk edge counts.
    Returns perm[global] -> new global id (same core, reassigned chunk)."""
    N, CORES, NPC, CH = cfg.N, cfg.CORES, cfg.NPC, cfg.CH
    deg = np.bincount(dst, minlength=N).astype(np.int64)
    perm = np.empty(N, np.int64)
    for c in range(CORES):
        d = deg[c * NPC:(c + 1) * NPC]
        order = np.argsort(-d, kind="stable")
        loads = np.zeros(CH, np.int64)
        counts = np.zeros(CH, np.int64)
        cap = np.full(CH, P, np.int64)
        cap[CH - 1] = NPC - (CH - 1) * P if NPC % P else P
        newloc = np.empty(NPC, np.int64)
        import heapq
        heap = [(0, k) for k in range(CH)]
        heapq.heapify(heap)
        for i in order:
            while True:
                l, k = heapq.heappop(heap)
                if counts[k] < cap[k]:
                    break
            newloc[i] = k * P + counts[k]
            counts[k] += 1
            loads[k] += d[i]
            if counts[k] < cap[k]:
                heapq.heappush(heap, (loads[k], k))
        perm[c * NPC:(c + 1) * NPC] = c * NPC + newloc
    return perm


def prep_inputs(cfg, x, edge_index, batch, W1, a_src1, a_dst1, W2, a_src2, a_dst2, fcW):
    N, CORES, NPC, CH = cfg.N, cfg.CORES, cfg.NPC, cfg.CH
    src = np.concatenate([edge_index[0], np.arange(N)]).astype(np.int64)
    dst = np.concatenate([edge_index[1], np.arange(N)]).astype(np.int64)
    perm = balance_perm(cfg, dst)
    src, dst = perm[src], perm[dst]
    inv = np.empty(N, np.int64)
    inv[perm] = np.arange(N)
    x = x[inv]
    batch = batch[inv]
    edges = prep_edges(cfg, src, dst)

    H = 2
    HID = cfg.HID
    rhs1 = np.zeros((cfg.IN, H * HID + 4), np.float32)
    rhs1[:, :H * HID] = W1
    for h in range(H):
        rhs1[:, H * HID + h] = W1[:, h * HID:(h + 1) * HID] @ a_src1[h]
        rhs1[:, H * HID + 2 + h] = W1[:, h * HID:(h + 1) * HID] @ a_dst1[h]
    rhs2 = np.zeros((H * HID, HID + 2), np.float32)
    rhs2[:, :HID] = W2
    rhs2[:, HID] = W2 @ a_src2[0]
    rhs2[:, HID + 1] = W2 @ a_dst2[0]

    iota128 = np.tile(np.arange(P, dtype=np.float32), (P, 1)).astype(NP_BF16)
    iota512 = np.tile(np.arange(cfg.G, dtype=np.float32), (P, 1))
    cnt = np.bincount(batch, minlength=cfg.G).astype(np.float32)
    invc = 1.0 / np.maximum(cnt, 1.0)
    invc_b = np.tile(invc, (HID, 1)).astype(np.float32)

    xT = np.zeros((cfg.IN, CORES * cfg.NCH), np.float32)
    gsl = np.full((CORES, cfg.NCH), 999.0, np.float32)
    for c in range(CORES):
        xT[:, c * cfg.NCH:c * cfg.NCH + NPC] = x[c * NPC:(c + 1) * NPC].T
        gsl[c, :NPC] = batch[c * NPC:(c + 1) * NPC]

    in_maps = []
    for c in range(CORES):
        in_maps.append(dict(
            xT=np.ascontiguousarray(xT[:, c * cfg.NCH:(c + 1) * cfg.NCH]),
            rhs1=rhs1, rhs2=rhs2, fcW=fcW.astype(np.float32),
            iota128=iota128, iota512=iota512, invc=invc_b,
            gslot=gsl[c].reshape(CH, P).T.copy(),
            **edges[c],
        ))
    return in_maps


# -------------------------------------------------------------- bass builder

def build_nc(cfg):
    N, CH, T, T_LO = cfg.N, cfg.CH, cfg.T, cfg.T_LO
    SEC_LO, SEC_HI, EC, NPC = cfg.SEC_LO, cfg.SEC_HI, cfg.EC, cfg.NPC
    HID, G = cfg.HID, cfg.G
    HALF = cfg.HALF
    R = list(range(cfg.CORES))

    NQ = int(os.environ.get("GATHER_QUEUES", "4"))
    nc = bacc.Bacc(num_swdge_queues=NQ)
    pi = lambda n, s, d=F32: nc.declare_dram_parameter(n, s, d, isOutput=False)
    xT = pi("xT", [cfg.IN, cfg.NCH])
    rhs1 = pi("rhs1", [cfg.IN, 132])
    rhs2 = pi("rhs2", [2 * HID, HID + 2])
    fcW = pi("fcW", [HID, 2])
    iota128 = pi("iota128", [P, P], BF16)
    iota512 = pi("iota512", [P, G])
    invc = pi("invc", [HID, G])
    gslot = pi("gslot", [P, CH])
    gl = pi("gl", [P, CH * SEC_LO // 16], I16)
    gh = pi("gh", [P, CH * SEC_HI // 16], I16)
    slot = pi("slot", [P, CH * T])
    s0t = pi("s0t", [P, CH * T * P], FP8)
    out_lg = nc.declare_dram_parameter("out_lg", [G, 2], F32, isOutput=True)

    shard1 = nc.dram_tensor("shard1", [NPC, ROW1], BF16)
    table1 = nc.dram_tensor("table1", [N, ROW1], BF16, addr_space="Shared")
    shard2 = nc.dram_tensor("shard2", [NPC, ROW2], BF16)
    table2 = nc.dram_tensor("table2", [N, ROW2], BF16, addr_space="Shared")
    pool_loc = nc.dram_tensor("pool_loc", [HID, G], F32)
    pool_sh = nc.dram_tensor("pool_sh", [HID, G], F32, addr_space="Shared")

    groups = [tuple(range(a, min(a + cfg.G_CH, CH))) for a in range(0, CH, cfg.G_CH)]

    # SWDGE descriptor-ring pacing, per queue: each queue runs on its own Q7
    # core pair (queue q -> cores 2q,2q+1), so gathers on different queues emit
    # concurrently; within a queue keep outstanding ring entries under budget
    # via probe-read cross-engine deps.
    gather_fifo = {q: [] for q in range(NQ)}
    gather_rr = [0]

    def paced_gather(probe_pool, **kw):
        q = gather_rr[0] % NQ
        gather_rr[0] += 1
        e = kw["num_idxs"] // 16 + 1
        inst = nc.gpsimd.dma_gather(single_packet=False, queue_num=q, **kw)
        gp_t = probe_pool.tile([1, 2], BF16, tag="gprobe", name="gprobe")
        rd = nc.vector.tensor_copy(out=gp_t[:], in_=kw["out_ap"][0:1, 0, 0:2])
        fifo = gather_fifo[q]
        tot = sum(x[1] for x in fifo) + e
        while fifo and (tot > 110 or len(fifo) >= 2):
            _, eo, rdo = fifo.pop(0)
            add_dep_helper(inst.ins, rdo.ins, sync=True, reason="swdge ring pacing")
            tot -= eo
        fifo.append((inst, e, rd))
        return inst

    with tile.TileContext(nc) as tc, ExitStack() as ctx:
        cp = ctx.enter_context(tc.tile_pool(name="const", bufs=1))
        dio = ctx.enter_context(tc.tile_pool(name="dio", bufs=3))
        dps = ctx.enter_context(tc.tile_pool(name="dps", bufs=2, space="PSUM"))
        o1p = ctx.enter_context(tc.tile_pool(name="o1p", bufs=1))
        ixp = ctx.enter_context(tc.tile_pool(name="ixp", bufs=2))
        gp = ctx.enter_context(tc.tile_pool(name="gp", bufs=4))
        stp = ctx.enter_context(tc.tile_pool(name="stp", bufs=2))
        sxp = ctx.enter_context(tc.tile_pool(name="sxp", bufs=4))
        xp = ctx.enter_context(tc.tile_pool(name="xp", bufs=3))
        ups = ctx.enter_context(tc.tile_pool(name="ups", bufs=2, space="PSUM"))
        aps = ctx.enter_context(tc.tile_pool(name="aps", bufs=1, space="PSUM"))
        pps = ctx.enter_context(tc.tile_pool(name="pps", bufs=1, space="PSUM"))
        fin = ctx.enter_context(tc.tile_pool(name="fin", bufs=3))

        io128 = cp.tile([P, P], BF16)
        nc.sync.dma_start(out=io128[:], in_=iota128[:])
        io512 = cp.tile([P, G], F32)
        nc.sync.dma_start(out=io512[:], in_=iota512[:])
        r1sb = cp.tile([cfg.IN, 132], F32)
        nc.sync.dma_start(out=r1sb[:], in_=rhs1[:])
        r2sb = cp.tile([2 * HID, HID + 2], F32)
        nc.sync.dma_start(out=r2sb[:], in_=rhs2[:])
        fcsb = cp.tile([HID, 2], F32)
        nc.sync.dma_start(out=fcsb[:], in_=fcW[:])
        icsb = cp.tile([HID, G], F32)
        nc.sync.dma_start(out=icsb[:], in_=invc[:])
        gssb = cp.tile([P, CH], F32)
        nc.sync.dma_start(out=gssb[:], in_=gslot[:])
        slsb = cp.tile([P, CH * T], F32)
        nc.sync.dma_start(out=slsb[:], in_=slot[:])
        idsb = cp.tile([P, P], F32)
        make_identity(nc, idsb[:])
        out1 = o1p.tile([P, CH * P], F32)
        adloc1 = cp.tile([P, CH, 2], BF16)  # per-chunk dst attention terms
        adloc2 = cp.tile([P, CH, 1], BF16)

        # ---------------- dense 1: rows of table1 ----------------
        for t in range(CH):
            nv = min(P, NPC - t * P)
            xt = dio.tile([P, P], F32, tag="xt")
            nc.sync.dma_start(out=xt[:], in_=xT[:, t * P:(t + 1) * P])
            ps = dps.tile([P, 132], F32, tag="dtmp")
            nc.tensor.matmul(out=ps[:], lhsT=xt[:], rhs=r1sb[:], start=True, stop=True)
            row = dio.tile([P, ROW1], BF16, tag="row1")
            nc.vector.tensor_copy(out=row[:, 0:64], in_=ps[:, 0:64])
            nc.vector.tensor_copy(out=row[:, 65:129], in_=ps[:, 64:128])
            nc.vector.memset(row[:, 64:65], 1.0)
            nc.vector.memset(row[:, 129:130], 1.0)
            nc.vector.tensor_copy(out=row[:, 130:132], in_=ps[:, 128:130])
            nc.vector.memset(row[:, 132:ROW1], 0.0)
            nc.vector.tensor_copy(out=adloc1[:, t, :], in_=ps[:, 130:132])
            nc.sync.dma_start(out=shard1[t * P:t * P + nv, :], in_=row[:nv, :])

        tc.strict_bb_all_engine_barrier()
        nc.gpsimd.collective_compute(
            "AllGather", ALU.bypass, replica_groups=[R],
            ins=[shard1[:]], outs=[table1[:]])

        # ---------------- edge phase (both layers) ----------------
        def edge_layer(tabA, tabB, adloc, row_w, nheads, finalize):
            for grp in groups:
                g0, ng = grp[0], len(grp)
                nlo, nhi = ng * SEC_LO, ng * SEC_HI
                glt = ixp.tile([P, nlo // 16], I16, tag="glt")
                nc.sync.dma_start(out=glt[:], in_=gl[:, g0 * SEC_LO // 16:(g0 * SEC_LO + nlo) // 16])
                ght = ixp.tile([P, nhi // 16], I16, tag="ght")
                nc.sync.dma_start(out=ght[:], in_=gh[:, g0 * SEC_HI // 16:(g0 * SEC_HI + nhi) // 16])
                stt = stp.tile([P, ng * T * P], FP8, tag="stt")
                nc.sync.dma_start(out=stt[:], in_=s0t[:, g0 * T * P:(g0 + ng) * T * P])
                hgl = gp.tile([P, nlo // P, row_w], BF16, tag="hgl")
                paced_gather(xp, out_ap=hgl[:], in_ap=tabA, idxs_ap=glt[:],
                             num_idxs=nlo, num_idxs_reg=nlo, elem_size=row_w)
                hgh = gp.tile([P, nhi // P, row_w], BF16, tag="hgh")
                paced_gather(xp, out_ap=hgh[:], in_ap=tabB, idxs_ap=ght[:],
                             num_idxs=nhi, num_idxs_reg=nhi, elem_size=row_w)
                for ci, c in enumerate(grp):
                    # per-edge ad via fp8 one-hot matmul against local ad chunk
                    adps = aps.tile([P, T, nheads], F32, tag="adps")
                    for t in range(T):
                        nc.tensor.matmul(
                            out=adps[:, t, :],
                            lhsT=stt[:, (ci * T + t) * P:(ci * T + t + 1) * P],
                            rhs=adloc[:, c, :], start=True, stop=True)
                    adsb = xp.tile([P, T, nheads], BF16, tag="adsb")
                    nc.vector.tensor_copy(out=adsb[:], in_=adps[:])
                    # s = as + ad ; x = exp(max(s, 0.2 s))
                    ssb = xp.tile([P, T, nheads], BF16, tag="ssb")
                    uo = 130 if nheads == 2 else 65
                    for sec, hg_t, t0, nt in ((0, hgl, 0, T_LO), (1, hgh, T_LO, T - T_LO)):
                        nc.vector.tensor_tensor(
                            out=ssb[:, t0:t0 + nt, :],
                            in0=hg_t[:, ci * nt:(ci + 1) * nt, uo:uo + nheads],
                            in1=adsb[:, t0:t0 + nt, :], op=ALU.add)
                    s2 = xp.tile([P, T, nheads], BF16, tag="s2")
                    nc.vector.tensor_scalar(out=s2[:], in0=ssb[:], scalar1=0.2,
                                            scalar2=None, op0=ALU.mult)
                    nc.vector.tensor_tensor(out=s2[:], in0=ssb[:], in1=s2[:], op=ALU.max)
                    xsb = xp.tile([P, T, nheads], F32, tag="xsb")
                    nc.scalar.activation(out=xsb[:], in_=s2[:], func=AF.Exp, scale=1.0)
                    # selection matmuls
                    Us = [ups.tile([P, 65], F32, tag=f"U{h}", name=f"U{h}") for h in range(nheads)]
                    for t in range(T):
                        if t < T_LO:
                            hg_t, tt, nt = hgl, t, T_LO
                        else:
                            hg_t, tt, nt = hgh, t - T_LO, T - T_LO
                        for h in range(nheads):
                            S = sxp.tile([P, P], BF16, tag=f"S{h}")
                            nc.vector.tensor_scalar(
                                out=S[:], in0=io128[:],
                                scalar1=slsb[:, c * T + t:c * T + t + 1],
                                scalar2=xsb[:, t, h:h + 1],
                                op0=ALU.is_equal, op1=ALU.mult)
                            nc.tensor.matmul(
                                out=Us[h][:], lhsT=S[:],
                                rhs=hg_t[:, ci * nt + tt, h * 65:(h + 1) * 65],
                                start=(t == 0), stop=(t == T - 1))
                    finalize(c, Us)

        def fin1(c, Us):
            den = fin.tile([P, 2], F32, tag="den1")
            rd = fin.tile([P, 2], F32, tag="rd1")
            for h in range(2):
                nc.vector.tensor_scalar(out=den[:, h:h + 1], in0=Us[h][:, 64:65],
                                        scalar1=1e-20, scalar2=None, op0=ALU.add)
            nc.vector.reciprocal(out=rd[:], in_=den[:])
            for h in range(2):
                nc.vector.tensor_scalar(
                    out=out1[:, c * P + h * 64:c * P + (h + 1) * 64],
                    in0=Us[h][:, 0:64], scalar1=rd[:, h:h + 1], scalar2=0.0,
                    op0=ALU.mult, op1=ALU.max)

        edge_layer(table1[0:HALF, :], table1[HALF:N, :], adloc1, ROW1, 2, fin1)

        # ---------------- dense 2 ----------------
        for t in range(CH):
            nv = min(P, NPC - t * P)
            tp = dps.tile([P, P], F32, tag="dtmp")
            nc.tensor.transpose(out=tp[:], in_=out1[:, t * P:(t + 1) * P], identity=idsb[:])
            h1T = dio.tile([P, P], F32, tag="h1T")
            nc.scalar.copy(out=h1T[:], in_=tp[:])
            ps = dps.tile([P, HID + 2], F32, tag="dtmp")
            nc.tensor.matmul(out=ps[:], lhsT=h1T[:], rhs=r2sb[:], start=True, stop=True)
            row = dio.tile([P, ROW2], BF16, tag="row2")
            nc.vector.tensor_copy(out=row[:, 0:64], in_=ps[:, 0:64])
            nc.vector.memset(row[:, 64:65], 1.0)
            nc.vector.tensor_copy(out=row[:, 65:66], in_=ps[:, 64:65])
            nc.vector.memset(row[:, 66:ROW2], 0.0)
            nc.vector.tensor_copy(out=adloc2[:, t, :], in_=ps[:, 65:66])
            nc.sync.dma_start(out=shard2[t * P:t * P + nv, :], in_=row[:nv, :])

        tc.strict_bb_all_engine_barrier()
        nc.gpsimd.collective_compute(
            "AllGather", ALU.bypass, replica_groups=[R],
            ins=[shard2[:]], outs=[table2[:]])

        # ---------------- edge layer 2 + pooling ----------------
        plT = pps.tile([HID, G], F32, name="plT")

        def fin2(c, Us):
            den = fin.tile([P, 1], F32, tag="den2")
            rd = fin.tile([P, 1], F32, tag="rd2")
            nc.vector.tensor_scalar(out=den[:], in0=Us[0][:, 64:65],
                                    scalar1=1e-20, scalar2=None, op0=ALU.add)
            nc.vector.reciprocal(out=rd[:], in_=den[:])
            o2 = fin.tile([P, HID], F32, tag="o2")
            nc.vector.tensor_scalar(out=o2[:], in0=Us[0][:, 0:64],
                                    scalar1=rd[:], scalar2=0.0,
                                    op0=ALU.mult, op1=ALU.max)
            sg = fin.tile([P, G], F32, tag="sg")
            nc.vector.tensor_scalar(out=sg[:], in0=io512[:],
                                    scalar1=gssb[:, c:c + 1], scalar2=None,
                                    op0=ALU.is_equal)
            nc.tensor.matmul(out=plT[:], lhsT=o2[:], rhs=sg[:],
                             start=(c == 0), stop=(c == CH - 1))

        edge_layer(table2[0:HALF, :], table2[HALF:N, :], adloc2, ROW2, 1, fin2)

        plsb = fin.tile([HID, G], F32)
        nc.vector.tensor_copy(out=plsb[:], in_=plT[:])
        nc.sync.dma_start(out=pool_loc[:], in_=plsb[:])
        tc.strict_bb_all_engine_barrier()
        nc.gpsimd.collective_compute(
            "AllReduce", ALU.add, replica_groups=[R],
            ins=[pool_loc[:]], outs=[pool_sh[:]])
        plr = fin.tile([HID, G], F32)
        nc.sync.dma_start(out=plr[:], in_=pool_sh[:])
        nc.vector.tensor_tensor(out=plr[:], in0=plr[:], in1=icsb[:], op=ALU.mult)
        for gt in range(max(1, G // P)):
            gw = min(P, G - gt * P)
            lg = dps.tile([P, 2], F32, tag="dtmp")
            nc.tensor.matmul(out=lg[:gw], lhsT=plr[:, gt * P:gt * P + gw], rhs=fcsb[:],
                             start=True, stop=True)
            mx = fin.tile([P, 1], F32, tag="mx")
            nc.vector.tensor_reduce(out=mx[:gw], in_=lg[:gw], op=ALU.max,
                                    axis=mybir.AxisListType.X)
            t1 = fin.tile([P, 2], F32, tag="t1")
            nc.vector.tensor_scalar(out=t1[:gw], in0=lg[:gw], scalar1=mx[:gw],
                                    scalar2=None, op0=ALU.subtract)
            ex = fin.tile([P, 2], F32, tag="ex")
            es = fin.tile([P, 1], F32, tag="es")
            nc.scalar.activation(out=ex[:gw], in_=t1[:gw], func=AF.Exp, accum_out=es[:gw])
            ln = fin.tile([P, 1], F32, tag="ln")
            nc.scalar.activation(out=ln[:gw], in_=es[:gw], func=AF.Ln)
            lsm = fin.tile([P, 2], F32, tag="lsm")
            nc.vector.tensor_scalar(out=lsm[:gw], in0=t1[:gw], scalar1=ln[:gw],
                                    scalar2=None, op0=ALU.subtract)
            nc.sync.dma_start(out=out_lg[gt * P:gt * P + gw, :], in_=lsm[:gw])

    nc.compile()
    return nc


# ------------------------------------------------------------------ entry

LAST_EXEC_NS = None

def kernel(x, edge_index, batch, W1, a_src1, a_dst1, b1, W2, a_src2, a_dst2, b2,
           fcW, fcb):
    x = np.asarray(x, np.float32)
    edge_index = np.asarray(edge_index, np.int64)
    batch = np.asarray(batch, np.int64)
    for b in (b1, b2, fcb):
        assert np.abs(np.asarray(b)).max() == 0.0, "nonzero bias unsupported"
    cfg = plan_cfg(N=x.shape[0], E0=edge_index.shape[1], G=512)
    in_maps = prep_inputs(cfg, x, edge_index, batch,
                          np.asarray(W1, np.float32), np.asarray(a_src1, np.float32),
                          np.asarray(a_dst1, np.float32), np.asarray(W2, np.float32),
                          np.asarray(a_src2, np.float32), np.asarray(a_dst2, np.float32),
                          np.asarray(fcW, np.float32))
    nc = build_nc(cfg)
    trace = os.environ.get("KERNEL_TRACE") == "1"
    res = run_bass_kernel_spmd(nc, in_maps, list(range(cfg.CORES)), trace=trace)
    global LAST_EXEC_NS
    LAST_EXEC_NS = res.exec_time_ns
    if trace:
        print(f"HW exec time: {res.exec_time_ns} ns "
              f"(mean {res.mean_exec_time_ns} ns)")
    return np.asarray(res.results[0]["out_lg"], np.float32)


# revision 10
# speedup vs baseline: 2.9220x; 1.1425x over previous
"""GAT classifier on 8 trn2 NeuronCores (Bass/Tile) — v2.

Sharding: 1D node partition (6250 nodes/core); edges assigned to the core
owning their dst node, sorted by dst into 128-node chunks.

v2 design (vs v1): the per-edge SWDGE gather count is halved and payloads
move to bf16. Per edge per layer there is exactly ONE gathered element:
  L1: 512B bf16 row [h0(64)|1|h1(64)|1|as0|as1|pad]
  L2: 256B bf16 row [h(64)|1|as|pad]
The dst-side attention term ad_e is produced without any gather: a host
-streamed fp8 one-hot S0T[j,e] (slot-of-edge) is matmul'd against the
local per-chunk ad table (PE, 2 cols), giving per-edge ad in PSUM.
Coefficients are computed directly: coef = exp(max(s, 0.2*s)), s = as+ad.
The segmented softmax + aggregation stays as PSUM-accumulated selection
matmuls with bf16 S matrices; denominators ride the baked-in "1" columns.
Layer tables are AllGathered between layers; pooling uses an AllReduce.
"""
import math
import os
import sys
from contextlib import ExitStack
from dataclasses import dataclass

import numpy as np

for _p in ("/opt/trn_rl_repo", "/root/.axon_site/_ro/trn_rl_repo"):
    if os.path.isdir(_p) and _p not in sys.path:
        sys.path.insert(0, _p)

import concourse.bacc as bacc
import concourse.bass as bass
import concourse.mybir as mybir
import concourse.tile as tile
from concourse.tile import add_dep_helper
from concourse.bass_utils import run_bass_kernel_spmd
from concourse.masks import make_identity

P = 128
AF = mybir.ActivationFunctionType
ALU = mybir.AluOpType
F32 = mybir.dt.float32
BF16 = mybir.dt.bfloat16
FP8 = mybir.dt.float8e4
I16 = mybir.dt.int16
NP_BF16 = mybir.dt.np(BF16)
NP_FP8 = mybir.dt.np(FP8)

ROW1 = 256   # L1 table row: bf16 slots [h0|1|h1|1|as0|as1|pad] -> 512B
ROW2 = 128   # L2 table row: bf16 slots [h|1|as|pad] -> 256B


@dataclass
class Cfg:
    N: int = 50000
    E0: int = 800000
    IN: int = 128
    HID: int = 64
    G: int = 512
    CORES: int = 8
    NPC: int = 0
    CH: int = 0
    HALF: int = 0
    SEC_LO: int = 0
    SEC_HI: int = 0
    EC: int = 0
    T: int = 0
    T_LO: int = 0
    G_CH: int = 2

    @property
    def NCH(self):  # padded per-core node count
        return self.CH * P


def plan_cfg(N, E0, G, CORES=8):
    c = Cfg(N=N, E0=E0, G=G, CORES=CORES)
    assert N % CORES == 0
    c.NPC = N // CORES
    c.CH = math.ceil(c.NPC / P)
    c.HALF = ((N // 2) + 127) & ~127
    assert c.HALF < 32768 * 2 and (N - c.HALF) <= 32767 and c.HALF <= 32767
    return c


# ----------------------------------------------------------------- host prep

def prep_edges(cfg, src, dst):
    """Per-core edge arrays. Returns list of dicts + fills cfg.SEC_*/EC/T."""
    owner = dst // cfg.NPC
    per_core = []
    maxlo = maxhi = 0
    for c in range(cfg.CORES):
        m = owner == c
        s = src[m]
        dl = dst[m] - c * cfg.NPC
        chunk = dl >> 7
        half = (s >= cfg.HALF).astype(np.int64)
        order = np.lexsort((s, half, chunk))
        s, dl, chunk, half = s[order], dl[order], chunk[order], half[order]
        key = chunk * 2 + half
        cnt = np.bincount(key, minlength=cfg.CH * 2).reshape(cfg.CH, 2)
        maxlo = max(maxlo, int(cnt[:, 0].max()))
        maxhi = max(maxhi, int(cnt[:, 1].max()))
        per_core.append((s, dl, chunk, half, cnt))
    cfg.SEC_LO = ((maxlo + 127) & ~127) or P
    cfg.SEC_HI = ((maxhi + 127) & ~127) or P
    cfg.EC = cfg.SEC_LO + cfg.SEC_HI
    cfg.T = cfg.EC // P
    cfg.T_LO = cfg.SEC_LO // P

    out = []
    for c in range(cfg.CORES):
        s, dl, chunk, half, cnt = per_core[c]
        gl = np.zeros((cfg.CH, cfg.SEC_LO), np.int16)
        gh = np.zeros((cfg.CH, cfg.SEC_HI), np.int16)
        sl = np.full((cfg.CH, cfg.EC), 300.0, np.float32)
        ofs = np.zeros(cfg.CH * 2 + 1, np.int64)
        np.cumsum(cnt.reshape(-1), out=ofs[1:])
        for k in range(cfg.CH):
            nlo, nhi = int(cnt[k, 0]), int(cnt[k, 1])
            a = ofs[2 * k]
            gl[k, :nlo] = s[a:a + nlo]
            sl[k, :nlo] = (dl[a:a + nlo] & 127).astype(np.float32)
            b = ofs[2 * k + 1]
            gh[k, :nhi] = s[b:b + nhi] - cfg.HALF
            sl[k, cfg.SEC_LO:cfg.SEC_LO + nhi] = (dl[b:b + nhi] & 127).astype(np.float32)

        def wrap16(a):  # idx i -> [i % 16, i // 16], replicated over 8 groups
            w = a.reshape(-1, 16).T.copy()
            return np.tile(w, (8, 1)).astype(np.int16)

        # S0T fp8 one-hot blocks: [128 j, CH*T*128] — col (c*T+t)*128+e is
        # one at row slot_e (pad slots 300 -> all-zero column).
        sl_t = sl.reshape(cfg.CH * cfg.T, P)  # [tile, e] slot values
        idx = sl_t.astype(np.int32)
        s0t = np.zeros((cfg.CH * cfg.T, P, P), NP_FP8)  # [tile, e, j]
        tt, ee = np.nonzero(idx < P)
        s0t[tt, ee, idx[tt, ee]] = 1.0
        s0e = np.ascontiguousarray(s0t.transpose(1, 0, 2).reshape(P, cfg.CH * cfg.T * P))
        s0t = np.ascontiguousarray(s0t.transpose(2, 0, 1).reshape(P, cfg.CH * cfg.T * P))

        out.append(dict(
            gl=wrap16(gl), gh=wrap16(gh),
            s0t=s0t, s0e=s0e,
        ))
    return out


def balance_perm(cfg, dst):
    """Per-core node->slot permutation equalizing per-chunk edge counts.
    Returns perm[global] -> new global id (same core, reassigned chunk)."""
    N, CORES, NPC, CH = cfg.N, cfg.CORES, cfg.NPC, cfg.CH
    deg = np.bincount(dst, minlength=N).astype(np.int64)
    perm = np.empty(N, np.int64)
    for c in range(CORES):
        d = deg[c * NPC:(c + 1) * NPC]
        order = np.argsort(-d, kind="stable")
        loads = np.zeros(CH, np.int64)
        counts = np.zeros(CH, np.int64)
        cap = np.full(CH, P, np.int64)
        cap[CH - 1] = NPC - (CH - 1) * P if NPC % P else P
        newloc = np.empty(NPC, np.int64)
        import heapq
        heap = [(0, k) for k in range(CH)]
        heapq.heapify(heap)
        for i in order:
            while True:
                l, k = heapq.heappop(heap)
                if counts[k] < cap[k]:
                    break
            newloc[i] = k * P + counts[k]
            counts[k] += 1
            loads[k] += d[i]
            if counts[k] < cap[k]:
                heapq.heappush(heap, (loads[k], k))
        perm[c * NPC:(c + 1) * NPC] = c * NPC + newloc
    return perm


def prep_inputs(cfg, x, edge_index, batch, W1, a_src1, a_dst1, W2, a_src2, a_dst2, fcW):
    N, CORES, NPC, CH = cfg.N, cfg.CORES, cfg.NPC, cfg.CH
    src = np.concatenate([edge_index[0], np.arange(N)]).astype(np.int64)
    dst = np.concatenate([edge_index[1], np.arange(N)]).astype(np.int64)
    perm = balance_perm(cfg, dst)
    src, dst = perm[src], perm[dst]
    inv = np.empty(N, np.int64)
    inv[perm] = np.arange(N)
    x = x[inv]
    batch = batch[inv]
    edges = prep_edges(cfg, src, dst)

    H = 2
    HID = cfg.HID
    rhs1 = np.zeros((cfg.IN, H * HID + 4), np.float32)
    rhs1[:, :H * HID] = W1
    for h in range(H):
        rhs1[:, H * HID + h] = W1[:, h * HID:(h + 1) * HID] @ a_src1[h]
        rhs1[:, H * HID + 2 + h] = W1[:, h * HID:(h + 1) * HID] @ a_dst1[h]
    rhs2 = np.zeros((H * HID, HID + 2), np.float32)
    rhs2[:, :HID] = W2
    rhs2[:, HID] = W2 @ a_src2[0]
    rhs2[:, HID + 1] = W2 @ a_dst2[0]

    iota512 = np.tile(np.arange(cfg.G, dtype=np.float32), (P, 1))
    cnt = np.bincount(batch, minlength=cfg.G).astype(np.float32)
    invc = 1.0 / np.maximum(cnt, 1.0)
    invc_b = np.tile(invc, (HID, 1)).astype(np.float32)

    xT = np.zeros((cfg.IN, CORES * cfg.NCH), np.float32)
    gsl = np.full((CORES, cfg.NCH), 999.0, np.float32)
    for c in range(CORES):
        xT[:, c * cfg.NCH:c * cfg.NCH + NPC] = x[c * NPC:(c + 1) * NPC].T
        gsl[c, :NPC] = batch[c * NPC:(c + 1) * NPC]

    in_maps = []
    for c in range(CORES):
        in_maps.append(dict(
            xT=np.ascontiguousarray(xT[:, c * cfg.NCH:(c + 1) * cfg.NCH]),
            rhs1=rhs1, rhs2=rhs2, fcW=fcW.astype(np.float32),
            iota512=iota512, invc=invc_b,
            gslot=gsl[c].reshape(CH, P).T.copy(),
            **edges[c],
        ))
    return in_maps


# -------------------------------------------------------------- bass builder

def build_nc(cfg):
    N, CH, T, T_LO = cfg.N, cfg.CH, cfg.T, cfg.T_LO
    SEC_LO, SEC_HI, EC, NPC = cfg.SEC_LO, cfg.SEC_HI, cfg.EC, cfg.NPC
    HID, G = cfg.HID, cfg.G
    HALF = cfg.HALF
    R = list(range(cfg.CORES))

    NQ = int(os.environ.get("GATHER_QUEUES", "4"))
    nc = bacc.Bacc(num_swdge_queues=NQ)
    pi = lambda n, s, d=F32: nc.declare_dram_parameter(n, s, d, isOutput=False)
    xT = pi("xT", [cfg.IN, cfg.NCH])
    rhs1 = pi("rhs1", [cfg.IN, 132])
    rhs2 = pi("rhs2", [2 * HID, HID + 2])
    fcW = pi("fcW", [HID, 2])
    iota512 = pi("iota512", [P, G])
    invc = pi("invc", [HID, G])
    gslot = pi("gslot", [P, CH])
    gl = pi("gl", [P, CH * SEC_LO // 16], I16)
    gh = pi("gh", [P, CH * SEC_HI // 16], I16)
    s0t = pi("s0t", [P, CH * T * P], FP8)
    s0e = pi("s0e", [P, CH * T * P], FP8)
    out_lg = nc.declare_dram_parameter("out_lg", [G, 2], F32, isOutput=True)

    shard1 = nc.dram_tensor("shard1", [NPC, ROW1], BF16)
    table1 = nc.dram_tensor("table1", [N, ROW1], BF16, addr_space="Shared")
    shard2 = nc.dram_tensor("shard2", [NPC, ROW2], BF16)
    table2 = nc.dram_tensor("table2", [N, ROW2], BF16, addr_space="Shared")
    pool_loc = nc.dram_tensor("pool_loc", [HID, G], F32)
    pool_sh = nc.dram_tensor("pool_sh", [HID, G], F32, addr_space="Shared")

    groups = [tuple(range(a, min(a + cfg.G_CH, CH))) for a in range(0, CH, cfg.G_CH)]

    # SWDGE descriptor-ring pacing, per queue: each queue runs on its own Q7
    # core pair (queue q -> cores 2q,2q+1), so gathers on different queues emit
    # concurrently; within a queue keep outstanding ring entries under budget
    # via probe-read cross-engine deps.
    gather_fifo = {q: [] for q in range(NQ)}
    gather_rr = [0]

    def paced_gather(probe_pool, **kw):
        q = gather_rr[0] % NQ
        gather_rr[0] += 1
        e = kw["num_idxs"] // 16 + 1
        inst = nc.gpsimd.dma_gather(single_packet=False, queue_num=q, **kw)
        gp_t = probe_pool.tile([1, 2], BF16, tag="gprobe", name="gprobe")
        rd = nc.vector.tensor_copy(out=gp_t[:], in_=kw["out_ap"][0:1, 0, 0:2])
        fifo = gather_fifo[q]
        tot = sum(x[1] for x in fifo) + e
        while fifo and (tot > 110 or len(fifo) >= 2):
            _, eo, rdo = fifo.pop(0)
            add_dep_helper(inst.ins, rdo.ins, sync=True, reason="swdge ring pacing")
            tot -= eo
        fifo.append((inst, e, rd))
        return inst

    with tile.TileContext(nc) as tc, ExitStack() as ctx:
        cp = ctx.enter_context(tc.tile_pool(name="const", bufs=1))
        dio = ctx.enter_context(tc.tile_pool(name="dio", bufs=3))
        dps = ctx.enter_context(tc.tile_pool(name="dps", bufs=2, space="PSUM"))
        o1p = ctx.enter_context(tc.tile_pool(name="o1p", bufs=1))
        ixp = ctx.enter_context(tc.tile_pool(name="ixp", bufs=2))
        gp = ctx.enter_context(tc.tile_pool(name="gp", bufs=4))
        stp = ctx.enter_context(tc.tile_pool(name="stp", bufs=2))
        sxp = ctx.enter_context(tc.tile_pool(name="sxp", bufs=4))
        xp = ctx.enter_context(tc.tile_pool(name="xp", bufs=3))
        ups = ctx.enter_context(tc.tile_pool(name="ups", bufs=2, space="PSUM"))
        aps = ctx.enter_context(tc.tile_pool(name="aps", bufs=1, space="PSUM"))
        pps = ctx.enter_context(tc.tile_pool(name="pps", bufs=1, space="PSUM"))
        fin = ctx.enter_context(tc.tile_pool(name="fin", bufs=3))

        io512 = cp.tile([P, G], F32)
        nc.sync.dma_start(out=io512[:], in_=iota512[:])
        r1sb = cp.tile([cfg.IN, 132], F32)
        nc.sync.dma_start(out=r1sb[:], in_=rhs1[:])
        r2sb = cp.tile([2 * HID, HID + 2], F32)
        nc.sync.dma_start(out=r2sb[:], in_=rhs2[:])
        fcsb = cp.tile([HID, 2], F32)
        nc.sync.dma_start(out=fcsb[:], in_=fcW[:])
        icsb = cp.tile([HID, G], F32)
        nc.sync.dma_start(out=icsb[:], in_=invc[:])
        gssb = cp.tile([P, CH], F32)
        nc.sync.dma_start(out=gssb[:], in_=gslot[:])
        idsb = cp.tile([P, P], F32)
        make_identity(nc, idsb[:])
        out1 = o1p.tile([P, CH * P], F32)
        adloc1 = cp.tile([P, CH, 2], BF16)  # per-chunk dst attention terms
        adloc2 = cp.tile([P, CH, 1], BF16)

        # ---------------- dense 1: rows of table1 ----------------
        for t in range(CH):
            nv = min(P, NPC - t * P)
            xt = dio.tile([P, P], F32, tag="xt")
            nc.sync.dma_start(out=xt[:], in_=xT[:, t * P:(t + 1) * P])
            ps = dps.tile([P, 132], F32, tag="dtmp")
            nc.tensor.matmul(out=ps[:], lhsT=xt[:], rhs=r1sb[:], start=True, stop=True)
            row = dio.tile([P, ROW1], BF16, tag="row1")
            nc.vector.tensor_copy(out=row[:, 0:64], in_=ps[:, 0:64])
            nc.vector.tensor_copy(out=row[:, 65:129], in_=ps[:, 64:128])
            nc.vector.memset(row[:, 64:65], 1.0)
            nc.vector.memset(row[:, 129:130], 1.0)
            nc.vector.tensor_copy(out=row[:, 130:132], in_=ps[:, 128:130])
            nc.vector.memset(row[:, 132:ROW1], 0.0)
            nc.vector.tensor_copy(out=adloc1[:, t, :], in_=ps[:, 130:132])
            nc.sync.dma_start(out=shard1[t * P:t * P + nv, :], in_=row[:nv, :])

        tc.strict_bb_all_engine_barrier()
        nc.gpsimd.collective_compute(
            "AllGather", ALU.bypass, replica_groups=[R],
            ins=[shard1[:]], outs=[table1[:]])

        # ---------------- edge phase (both layers) ----------------
        def edge_layer(tabA, tabB, adloc, row_w, nheads, finalize):
            for grp in groups:
                g0, ng = grp[0], len(grp)
                nlo, nhi = ng * SEC_LO, ng * SEC_HI
                glt = ixp.tile([P, nlo // 16], I16, tag="glt")
                nc.sync.dma_start(out=glt[:], in_=gl[:, g0 * SEC_LO // 16:(g0 * SEC_LO + nlo) // 16])
                ght = ixp.tile([P, nhi // 16], I16, tag="ght")
                nc.sync.dma_start(out=ght[:], in_=gh[:, g0 * SEC_HI // 16:(g0 * SEC_HI + nhi) // 16])
                stt = stp.tile([P, ng * T * P], FP8, tag="stt")
                nc.sync.dma_start(out=stt[:], in_=s0t[:, g0 * T * P:(g0 + ng) * T * P])
                set_ = stp.tile([P, ng * T * P], FP8, tag="set")
                nc.sync.dma_start(out=set_[:], in_=s0e[:, g0 * T * P:(g0 + ng) * T * P])
                hgl = gp.tile([P, nlo // P, row_w], BF16, tag="hgl")
                paced_gather(xp, out_ap=hgl[:], in_ap=tabA, idxs_ap=glt[:],
                             num_idxs=nlo, num_idxs_reg=nlo, elem_size=row_w)
                hgh = gp.tile([P, nhi // P, row_w], BF16, tag="hgh")
                paced_gather(xp, out_ap=hgh[:], in_ap=tabB, idxs_ap=ght[:],
                             num_idxs=nhi, num_idxs_reg=nhi, elem_size=row_w)
                for ci, c in enumerate(grp):
                    # per-edge ad via fp8 one-hot matmul against local ad chunk
                    adps = aps.tile([P, T, nheads], F32, tag="adps")
                    for t in range(T):
                        nc.tensor.matmul(
                            out=adps[:, t, :],
                            lhsT=stt[:, (ci * T + t) * P:(ci * T + t + 1) * P],
                            rhs=adloc[:, c, :], start=True, stop=True)
                    adsb = xp.tile([P, T, nheads], BF16, tag="adsb")
                    nc.vector.tensor_copy(out=adsb[:], in_=adps[:])
                    # s = as + ad ; x = exp(max(s, 0.2 s))
                    ssb = xp.tile([P, T, nheads], BF16, tag="ssb")
                    uo = 130 if nheads == 2 else 65
                    for sec, hg_t, t0, nt in ((0, hgl, 0, T_LO), (1, hgh, T_LO, T - T_LO)):
                        nc.vector.tensor_tensor(
                            out=ssb[:, t0:t0 + nt, :],
                            in0=hg_t[:, ci * nt:(ci + 1) * nt, uo:uo + nheads],
                            in1=adsb[:, t0:t0 + nt, :], op=ALU.add)
                    s2 = xp.tile([P, T, nheads], BF16, tag="s2")
                    nc.vector.tensor_scalar(out=s2[:], in0=ssb[:], scalar1=0.2,
                                            scalar2=None, op0=ALU.mult)
                    nc.vector.tensor_tensor(out=s2[:], in0=ssb[:], in1=s2[:], op=ALU.max)
                    xsb = xp.tile([P, T, nheads], F32, tag="xsb")
                    nc.scalar.activation(out=xsb[:], in_=s2[:], func=AF.Exp, scale=1.0)
                    # selection matmuls
                    Us = [ups.tile([P, 65], F32, tag=f"U{h}", name=f"U{h}") for h in range(nheads)]
                    for t in range(T):
                        if t < T_LO:
                            hg_t, tt, nt = hgl, t, T_LO
                        else:
                            hg_t, tt, nt = hgh, t - T_LO, T - T_LO
                        s0sl = set_[:, (ci * T + t) * P:(ci * T + t + 1) * P]
                        for h in range(nheads):
                            rs = sxp.tile([P, 65], BF16, tag=f"rs{h}")
                            nc.vector.tensor_scalar(
                                out=rs[:], in0=hg_t[:, ci * nt + tt, h * 65:(h + 1) * 65],
                                scalar1=xsb[:, t, h:h + 1], scalar2=None, op0=ALU.mult)
                            nc.tensor.matmul(
                                out=Us[h][:], lhsT=s0sl, rhs=rs[:],
                                start=(t == 0), stop=(t == T - 1))
                    finalize(c, Us)

        def fin1(c, Us):
            den = fin.tile([P, 2], F32, tag="den1")
            rd = fin.tile([P, 2], F32, tag="rd1")
            for h in range(2):
                nc.vector.tensor_scalar(out=den[:, h:h + 1], in0=Us[h][:, 64:65],
                                        scalar1=1e-20, scalar2=None, op0=ALU.add)
            nc.vector.reciprocal(out=rd[:], in_=den[:])
            for h in range(2):
                nc.vector.tensor_scalar(
                    out=out1[:, c * P + h * 64:c * P + (h + 1) * 64],
                    in0=Us[h][:, 0:64], scalar1=rd[:, h:h + 1], scalar2=0.0,
                    op0=ALU.mult, op1=ALU.max)
            # fused dense-2 for this chunk
            nv = min(P, NPC - c * P)
            tp = dps.tile([P, P], F32, tag="dtmp")
            nc.tensor.transpose(out=tp[:], in_=out1[:, c * P:(c + 1) * P], identity=idsb[:])
            h1T = dio.tile([P, P], F32, tag="h1T")
            nc.scalar.copy(out=h1T[:], in_=tp[:])
            ps = dps.tile([P, HID + 2], F32, tag="dtmp")
            nc.tensor.matmul(out=ps[:], lhsT=h1T[:], rhs=r2sb[:], start=True, stop=True)
            row = dio.tile([P, ROW2], BF16, tag="row2")
            nc.vector.tensor_copy(out=row[:, 0:64], in_=ps[:, 0:64])
            nc.vector.memset(row[:, 64:65], 1.0)
            nc.vector.tensor_copy(out=row[:, 65:66], in_=ps[:, 64:65])
            nc.vector.memset(row[:, 66:ROW2], 0.0)
            nc.vector.tensor_copy(out=adloc2[:, c, :], in_=ps[:, 65:66])
            nc.sync.dma_start(out=shard2[c * P:c * P + nv, :], in_=row[:nv, :])

        edge_layer(table1[0:HALF, :], table1[HALF:N, :], adloc1, ROW1, 2, fin1)

        tc.strict_bb_all_engine_barrier()
        nc.gpsimd.collective_compute(
            "AllGather", ALU.bypass, replica_groups=[R],
            ins=[shard2[:]], outs=[table2[:]])

        # ---------------- edge layer 2 + pooling ----------------
        plT = pps.tile([HID, G], F32, name="plT")

        def fin2(c, Us):
            den = fin.tile([P, 1], F32, tag="den2")
            rd = fin.tile([P, 1], F32, tag="rd2")
            nc.vector.tensor_scalar(out=den[:], in0=Us[0][:, 64:65],
                                    scalar1=1e-20, scalar2=None, op0=ALU.add)
            nc.vector.reciprocal(out=rd[:], in_=den[:])
            o2 = fin.tile([P, HID], F32, tag="o2")
            nc.vector.tensor_scalar(out=o2[:], in0=Us[0][:, 0:64],
                                    scalar1=rd[:], scalar2=0.0,
                                    op0=ALU.mult, op1=ALU.max)
            sg = fin.tile([P, G], F32, tag="sg")
            nc.vector.tensor_scalar(out=sg[:], in0=io512[:],
                                    scalar1=gssb[:, c:c + 1], scalar2=None,
                                    op0=ALU.is_equal)
            nc.tensor.matmul(out=plT[:], lhsT=o2[:], rhs=sg[:],
                             start=(c == 0), stop=(c == CH - 1))

        edge_layer(table2[0:HALF, :], table2[HALF:N, :], adloc2, ROW2, 1, fin2)

        plsb = fin.tile([HID, G], F32)
        nc.vector.tensor_copy(out=plsb[:], in_=plT[:])
        nc.sync.dma_start(out=pool_loc[:], in_=plsb[:])
        tc.strict_bb_all_engine_barrier()
        nc.gpsimd.collective_compute(
            "AllReduce", ALU.add, replica_groups=[R],
            ins=[pool_loc[:]], outs=[pool_sh[:]])
        plr = fin.tile([HID, G], F32)
        nc.sync.dma_start(out=plr[:], in_=pool_sh[:])
        nc.vector.tensor_tensor(out=plr[:], in0=plr[:], in1=icsb[:], op=ALU.mult)
        for gt in range(max(1, G // P)):
            gw = min(P, G - gt * P)
            lg = dps.tile([P, 2], F32, tag="dtmp")
            nc.tensor.matmul(out=lg[:gw], lhsT=plr[:, gt * P:gt * P + gw], rhs=fcsb[:],
                             start=True, stop=True)
            mx = fin.tile([P, 1], F32, tag="mx")
            nc.vector.tensor_reduce(out=mx[:gw], in_=lg[:gw], op=ALU.max,
                                    axis=mybir.AxisListType.X)
            t1 = fin.tile([P, 2], F32, tag="t1")
            nc.vector.tensor_scalar(out=t1[:gw], in0=lg[:gw], scalar1=mx[:gw],
                                    scalar2=None, op0=ALU.subtract)
            ex = fin.tile([P, 2], F32, tag="ex")
            es = fin.tile([P, 1], F32, tag="es")
            nc.scalar.activation(out=ex[:gw], in_=t1[:gw], func=AF.Exp, accum_out=es[:gw])
            ln = fin.tile([P, 1], F32, tag="ln")
            nc.scalar.activation(out=ln[:gw], in_=es[:gw], func=AF.Ln)
            lsm = fin.tile([P, 2], F32, tag="lsm")
            nc.vector.tensor_scalar(out=lsm[:gw], in0=t1[:gw], scalar1=ln[:gw],
                                    scalar2=None, op0=ALU.subtract)
            nc.sync.dma_start(out=out_lg[gt * P:gt * P + gw, :], in_=lsm[:gw])

    nc.compile()
    return nc


# ------------------------------------------------------------------ entry

LAST_EXEC_NS = None

def kernel(x, edge_index, batch, W1, a_src1, a_dst1, b1, W2, a_src2, a_dst2, b2,
           fcW, fcb):
    x = np.asarray(x, np.float32)
    edge_index = np.asarray(edge_index, np.int64)
    batch = np.asarray(batch, np.int64)
    for b in (b1, b2, fcb):
        assert np.abs(np.asarray(b)).max() == 0.0, "nonzero bias unsupported"
    cfg = plan_cfg(N=x.shape[0], E0=edge_index.shape[1], G=512)
    in_maps = prep_inputs(cfg, x, edge_index, batch,
                          np.asarray(W1, np.float32), np.asarray(a_src1, np.float32),
                          np.asarray(a_dst1, np.float32), np.asarray(W2, np.float32),
                          np.asarray(a_src2, np.float32), np.asarray(a_dst2, np.float32),
                          np.asarray(fcW, np.float32))
    nc = build_nc(cfg)
    trace = os.environ.get("KERNEL_TRACE") == "1"
    res = run_bass_kernel_spmd(nc, in_maps, list(range(cfg.CORES)), trace=trace)
    global LAST_EXEC_NS
    LAST_EXEC_NS = res.exec_time_ns
    if trace:
        print(f"HW exec time: {res.exec_time_ns} ns "
              f"(mean {res.mean_exec_time_ns} ns)")
    return np.asarray(res.results[0]["out_lg"], np.float32)
